# revision 37
# baseline (speedup 1.0000x reference)
"""Trainium2 Bass kernel for nn_AbilityGammaAttention.

Reference computation (per batch b):
    ws = s_j @ Ws_w.T + Ws_b                      # (P, A)
    uh = exp_tokens @ U_w.T                       # (Q, LE, A)
    e[q,p,t] = v . tanh(uh[q,t,:] + ws[p,:])      # (Q, P, LE)
    e masked by exp_mask (tokens), joint softmax over (Q, LE) per (b, p)
    out[q,p,:] = sum_t a[q,p,t] * exp_tokens[q,t,:], zeroed where req_mask[p]==0

Sharding: data-parallel over B across the 8 NeuronCores (batch b -> core b).

Design (v2 — separable ridge expansion instead of per-p tanh):
  The per-p ScalarE tanh over P*T*A elements (the v1 bottleneck, ~75us) is
  replaced by the separable approximation
      tanh(u + w) ~= c0(w) + cl(w)*u + sum_r cr(w)*tanh(ar*u + br)
                     + sum_j dj(w)*clamp(u, lo_j, hi_j)
  where u = uh[t,a] and w = ws[p,a].  The u-side basis is computed ONCE per
  token (R_s ScalarE tanh passes + R_v DVE clamp ops over [A, T]), and all the
  w-side structure collapses into small per-batch coefficient matrices
  G_k[a,p] = v_a * c_k(ws[p,a]) computed on the HOST (ws is host-computable
  from s_j/Ws_w).  e[:, t] then accumulates as NB matmuls [A,pa]^T @ [A,T]
  into PSUM — PE work independent of P.  The constant c0 folds into the
  per-partition exp bias.

  Other structure:
  - Host token compaction per (b,q) (as v1): unmasked tokens packed to the
    front, le = max count rounded up to 8.  Padded slots keep x=0 so they
    cannot touch the output; an additive (m-1)*1e9 rank-1 matmul masks them
    out of the softmax exactly.
  - Host req_mask compaction over p: only active p rows (padded to pa) get
    coefficients / output rows; host scatters into the zeroed full output.
  - Softmax normalization on the HOST: the device ships unnormalized
    out_raw[q,p,:] = sum_t exp(e-bound)*x and the per-chunk denominator
    accumulators (sums8); host divides.  This removes the global-Z join from
    the device pipeline, so apply/evacuation pipeline behind the main loop.
  - Everything streams in bf16 (x, xT, basis, G, a); matmuls run 1 cyc/row.
  - x is passed in BOTH layouts from the host (natural for the apply matmul,
    d-major transposed for the uh matmul) to keep PE free of transposes.
"""

import sys

if "/opt/trn_rl_repo" not in sys.path:
    sys.path.insert(0, "/opt/trn_rl_repo")

import numpy as np
import ml_dtypes

import concourse.bacc as bacc
import concourse.mybir as mybir
from concourse.masks import make_identity
from concourse.tile import TileContext

F32 = mybir.dt.float32
BF16 = mybir.dt.bfloat16
I32 = mybir.dt.int32
AF = mybir.ActivationFunctionType
ALU = mybir.AluOpType
NPBF16 = ml_dtypes.bfloat16

B, Q, LE, D, P, A = 8, 32, 128, 512, 32, 128
N_CORES = 8
DC = D // 128

# ---- ridge-basis parameters (offline fit, see session notes) -------------
# tanh(u+w) ~= c0(w) + cl(w)*u + sum_r cr(w) tanh(ALPHA_r u + BETA_r)
#            + sum_j dj(w) clamp(u, CLO_j, CHI_j)
ALPHA = [0.81614, 0.75456, 1.00156, 0.7428, 0.98255, 0.75628, 0.90041]
BETA = [-3.08039, -2.15134, -0.83113, 0.01514, 1.27361, 2.15807, 3.45762]
CLO = [-2.97443, 1.02768]
CHI = [-1.13153, 2.70644]
USE_LINEAR = True

_NG = 1201
_GRID = np.linspace(-6.5, 6.5, _NG)
_WGT = np.exp(-0.5 * _GRID**2) + 0.003


def _phi_of(grid):
    cols = [np.ones_like(grid)]
    if USE_LINEAR:
        cols.append(grid)
    for a_, b_ in zip(ALPHA, BETA):
        cols.append(np.tanh(a_ * grid + b_))
    for l_, h_ in zip(CLO, CHI):
        cols.append(np.clip(grid, l_, h_))
    return np.stack(cols, axis=0)  # (K, NG)


def _solve_matrices():
    Phi = _phi_of(_GRID)
    W = _WGT / _WGT.sum()
    Gm = (Phi * W) @ Phi.T
    Gm += 1e-9 * np.trace(Gm) / len(Gm) * np.eye(len(Gm))
    Gi = np.linalg.inv(Gm)
    M = Gi @ (Phi * W)
    phi0 = _phi_of(np.zeros(1))[:, 0]
    Kv = Gi @ phi0 / (phi0 @ Gi @ phi0)
    return M, phi0, Kv


_SOLVE_M, _PHI0, _KV = _solve_matrices()


def coeffs_for_w(w_flat):
    """c_k(w) for each w: weighted LS on the u-grid, constrained so the
    expansion is EXACT at u=0 (pads then correct on the host)."""
    Y = np.tanh(_GRID[:, None].astype(np.float32) + w_flat[None, :].astype(np.float32))
    C = _SOLVE_M.astype(np.float32) @ Y
    viol = np.tanh(w_flat.astype(np.float32)) - _PHI0.astype(np.float32) @ C
    return C + _KV.astype(np.float32)[:, None] * viol[None, :]


def build_kernel(q=Q, le=LE, pa=P):
    """Per-core kernel. q multiple of 4, le multiple of 8, pa multiple of 4."""
    T = q * le
    T2 = T // 2
    GW = 4 * le           # tokens per uh-group (4 q)
    n_t = len(ALPHA)
    n_c = len(CLO)
    NB = (1 if USE_LINEAR else 0) + n_t + n_c   # PE basis matmuls (excl mask)
    NCH = q // 4          # e-chunks (one per uh-group)
    assert le % 8 == 0 and q % 8 == 0 and pa % 4 == 0 and 4 * pa <= 128

    nc = bacc.Bacc("TRN2", target_bir_lowering=False, debug=False)

    xn_dram = nc.dram_tensor("x_nat", [le, q * D], BF16, kind="ExternalInput")
    xt_dram = nc.dram_tensor("x_t", [128, DC * T], BF16, kind="ExternalInput")
    uwt_dram = nc.dram_tensor("uw_t", [128, DC * A], BF16, kind="ExternalInput")
    g_dram = nc.dram_tensor("g_all", [A, NB * pa], BF16, kind="ExternalInput")
    eb_dram = nc.dram_tensor("ebias", [pa, 1], F32, kind="ExternalInput")
    out_dram = nc.dram_tensor("o_raw", [(q // 4) * 116, D], F32, kind="ExternalOutput")
    sums_dram = nc.dram_tensor("sums8", [pa, NCH], F32, kind="ExternalOutput")

    with TileContext(nc) as tc:
        with tc.tile_pool(name="live", bufs=1) as L:
            identf = L.tile([128, 128], F32)
            identb = L.tile([128, 128], BF16)
            xn_sb = L.tile([le, q * D], BF16)
            xt_sb = L.tile([128, DC * T], BF16)
            uwt_sb = L.tile([128, DC * A], BF16)
            g_sb = L.tile([A, NB * pa], BF16)
            eb_sb = L.tile([pa, 1], F32)
            NR = 4                       # basis regions (finer => better overlap)
            RW = T // NR
            GPR = NCH // NR              # 4-q groups per region
            uhq = [L.tile([A, RW], BF16, name=f"uhq{i}") for i in range(NR)]
            e_full = L.tile([pa, T], BF16)
            aT_all = L.tile([le, q * pa], BF16)
            sums8 = L.tile([pa, NCH], F32)

            # param DMAs first: nothing upstream of them
            nc.sync.dma_start(uwt_sb[:], uwt_dram[:])
            nc.sync.dma_start(g_sb[:], g_dram[:])
            nc.sync.dma_start(eb_sb[:], eb_dram[:])

            make_identity(nc, identf)
            nc.vector.tensor_copy(identb[:], identf[:])
            btab = L.tile([128, n_t], F32)
            for r in range(n_t):
                nc.gpsimd.memset(btab[:, r:r + 1], float(BETA[r]))

            with (
                tc.tile_pool(name="bas", bufs=1) as BP,
                tc.tile_pool(name="out", bufs=1) as OP,
                tc.tile_pool(name="ps", bufs=1, space="PSUM") as PS,
            ):
                # ---- load x (both layouts): all input DMAs up front -----
                # finer slices first so the first uh group starts early
                T4 = T // 4
                for qt in range(4):
                    for c in range(DC):
                        nc.sync.dma_start(
                            xt_sb[:, c * T + qt * T4: c * T + (qt + 1) * T4],
                            xt_dram[:, c * T + qt * T4: c * T + (qt + 1) * T4],
                        )
                for h in range(4):
                    nc.sync.dma_start(
                        xn_sb[:, h * (q // 4) * D:(h + 1) * (q // 4) * D],
                        xn_dram[:, h * (q // 4) * D:(h + 1) * (q // 4) * D],
                    )
                # ---- uh stream: all groups before the basis/accum loop --
                for g0 in range(NCH):
                    ups = PS.tile([A, GW], F32, tag="ups", bufs=2)
                    for c in range(DC):
                        nc.tensor.matmul(
                            ups[:],
                            uwt_sb[:, c * A:(c + 1) * A],
                            xt_sb[:, c * T + g0 * GW: c * T + (g0 + 1) * GW],
                            start=(c == 0), stop=(c == DC - 1),
                        )
                    nc.vector.tensor_copy(
                        uhq[g0 // GPR][:, (g0 % GPR) * GW:(g0 % GPR + 1) * GW],
                        ups[:])

                for reg in range(NR):
                    uhr = uhq[reg]
                    # ---- u-basis for this region (ScalarE + DVE) --------
                    bts = []
                    for r in range(n_t):
                        bt = BP.tile([A, RW], BF16, tag=f"bt{r}", bufs=2)
                        nc.scalar.activation(
                            bt[:], uhr[:], AF.Tanh,
                            bias=btab[:, r:r + 1], scale=float(ALPHA[r]),
                        )
                        bts.append(bt)
                    bcs = []
                    for j in range(n_c):
                        bc = BP.tile([A, RW], BF16, tag=f"bc{j}", bufs=2)
                        nc.vector.tensor_scalar(
                            bc[:], uhr[:],
                            scalar1=float(CLO[j]), scalar2=float(CHI[j]),
                            op0=ALU.max, op1=ALU.min,
                        )
                        bcs.append(bc)

                    # ---- e accumulation + exp + apply, per 4-q chunk ----
                    for gi in range(GPR):
                        g0 = reg * GPR + gi
                        cols = slice(g0 * GW, (g0 + 1) * GW)
                        lcol = slice(gi * GW, (gi + 1) * GW)
                        eps = PS.tile([pa, GW], F32, tag="eps", bufs=3)
                        kb = 0
                        if USE_LINEAR:
                            nc.tensor.matmul(
                                eps[:], g_sb[:, 0:pa], uhr[:, lcol],
                                start=True, stop=False,
                            )
                            kb = 1
                        for r in range(n_t):
                            nc.tensor.matmul(
                                eps[:], g_sb[:, (kb + r) * pa:(kb + r + 1) * pa],
                                bts[r][:, lcol], start=(kb == 0 and r == 0),
                                stop=False,
                            )
                        for j in range(n_c):
                            nc.tensor.matmul(
                                eps[:],
                                g_sb[:, (kb + n_t + j) * pa:(kb + n_t + j + 1) * pa],
                                bcs[j][:, lcol], start=False,
                                stop=(j == n_c - 1),
                            )
                        nc.scalar.activation(
                            e_full[:, cols], eps[:], AF.Exp,
                            bias=eb_sb[:, 0:1], scale=1.0,
                            accum_out=sums8[:, g0:g0 + 1],
                        )

                        # aT transposes for the 4 q of this chunk
                        atp = PS.tile([le, 4 * pa], BF16, tag="atp", bufs=1)
                        for k in range(4):
                            iq = g0 * 4 + k
                            nc.tensor.transpose(
                                atp[:, k * pa:(k + 1) * pa],
                                e_full[0:pa, iq * le:(iq + 1) * le],
                                identb[0:pa, 0:pa],
                            )
                        nc.vector.tensor_copy(
                            aT_all[:, g0 * 4 * pa:(g0 + 1) * 4 * pa], atp[:])

                        # apply matmuls: 2 q per PSUM tile at bases {0, 32};
                        # both pairs land in one osb tile -> one DMA per chunk
                        osb = OP.tile([116, D], F32, tag="osb", bufs=3)
                        for pr in range(2):
                            ops = PS.tile([64, D], F32, tag="ops", bufs=2)
                            for k in range(2):
                                iq = g0 * 4 + pr * 2 + k
                                nc.tensor.matmul(
                                    ops[k * 32:k * 32 + pa, :],
                                    aT_all[:, iq * pa:(iq + 1) * pa],
                                    xn_sb[:, iq * D:(iq + 1) * D],
                                    start=True, stop=True,
                                )
                            dst = osb[pr * 64:pr * 64 + 52, :]
                            if reg == NR - 1:
                                nc.scalar.activation(dst, ops[0:52, :], AF.Copy,
                                                     bias=0.0, scale=1.0)
                            else:
                                nc.vector.tensor_copy(dst, ops[0:52, :])
                        nc.sync.dma_start(
                            out_dram[g0 * 116:(g0 + 1) * 116, :], osb[:])

                nc.sync.dma_start(sums_dram[:], sums8[:])

    nc.compile()
    return nc


_NC_CACHE = {}
LAST_NC = None


def _get_nc(q=Q, le=LE, pa=P):
    key = (q, le, pa)
    if key not in _NC_CACHE:
        _NC_CACHE[key] = build_kernel(q, le, pa)
    return _NC_CACHE[key]


def _compact_tokens(exp_tokens, exp_mask, le):
    """Per-(b,q) host compaction. Returns x_c (b,q,le,D) f32 and m_c (b,q,le)."""
    b, q, full, d = exp_tokens.shape
    x_c = np.zeros((b, q, le, d), dtype=np.float32)
    m_c = np.zeros((b, q, le), dtype=np.float32)
    for bi in range(b):
        for qi in range(q):
            idx = np.flatnonzero(exp_mask[bi, qi])
            n = len(idx)
            x_c[bi, qi, :n] = exp_tokens[bi, qi, idx]
            m_c[bi, qi, :n] = 1.0
    return x_c, m_c


def kernel(exp_tokens, exp_mask, s_j, req_mask, Ws_w, Ws_b, U_w, v_w):
    """Full-input entry point: shard over B across 8 cores, gather output."""
    from concourse.bass_utils import run_bass_kernel_spmd

    exp_tokens = np.asarray(exp_tokens, dtype=np.float32)
    exp_mask = np.asarray(exp_mask, dtype=np.int32)
    s_j = np.asarray(s_j, dtype=np.float32)
    req_mask = np.asarray(req_mask, dtype=np.int32)
    Ws_w = np.asarray(Ws_w, dtype=np.float32)
    Ws_b = np.asarray(Ws_b, dtype=np.float32)
    U_w = np.asarray(U_w, dtype=np.float32)
    v_w = np.asarray(v_w, dtype=np.float32)

    counts = exp_mask.sum(axis=2)
    le = int(min(LE, max(64, -(-int(counts.max()) // 8) * 8)))
    x_c, m_c = _compact_tokens(exp_tokens, exp_mask, le)

    p_counts = req_mask.sum(axis=1)
    pa = int(min(P, max(4, -(-int(p_counts.max()) // 4) * 4)))

    bound = float(np.abs(v_w).sum()) + 1.0
    n_t, n_c = len(ALPHA), len(CLO)
    NB = (1 if USE_LINEAR else 0) + n_t + n_c

    # host-side w-branch: ws, coefficients, G matrices
    ws = (s_j.astype(np.float64) @ Ws_w.T.astype(np.float64)
          + Ws_b.astype(np.float64)).astype(np.float32)      # (B, P, A)
    vrow = v_w[0]                                            # (A,)

    T = Q * le
    # device basis values at u=0 (bf16-rounded, replicating device tiles)
    phi0_dev = np.zeros(NB, dtype=np.float32)                # excl constant
    k0 = 0
    if USE_LINEAR:
        phi0_dev[0] = 0.0
        k0 = 1
    for r in range(n_t):
        phi0_dev[k0 + r] = np.float32(np.tanh(BETA[r])).astype(NPBF16).astype(np.float32)
    for j in range(n_c):
        phi0_dev[k0 + n_t + j] = np.float32(np.clip(0.0, CLO[j], CHI[j])
                                            ).astype(NPBF16).astype(np.float32)

    uw_t = np.ascontiguousarray(
        U_w.reshape(A, DC, 128).transpose(2, 1, 0).reshape(128, DC * A)
    ).astype(NPBF16)

    in_maps = []
    pidx_all = []
    a_pad_all = []
    npad_all = []
    for b in range(N_CORES):
        pidx = np.flatnonzero(req_mask[b])
        pidx_all.append(pidx)
        ws_act = np.zeros((pa, A), dtype=np.float32)
        ws_act[:len(pidx)] = ws[b, pidx]
        C = coeffs_for_w(ws_act.reshape(-1)).reshape(-1, pa, A)  # (K, pa, A)
        # zero out padded p rows entirely
        if len(pidx) < pa:
            C[:, len(pidx):, :] = 0.0
        g_all = np.zeros((A, NB * pa), dtype=np.float32)
        for k in range(NB):
            g_all[:, k * pa:(k + 1) * pa] = (C[1 + k] * vrow[None, :]).T
        g_bf = g_all.astype(NPBF16)
        c0s = (C[0] * vrow[None, :]).sum(axis=1).astype(np.float32)  # (pa,)
        ebias = c0s - bound

        # padded slots (u = 0): their device e and exp, for host Z-correction
        gb = g_bf.astype(np.float32)
        e_pad = c0s.copy()
        for k in range(NB):
            e_pad += gb[:, k * pa:(k + 1) * pa].sum(axis=0) * phi0_dev[k]
        a_pad_all.append(np.exp(e_pad.astype(np.float64) - bound))
        npad_all.append(float(le * Q - int(m_c[b].sum())))

        xb = x_c[b]                                          # (Q, le, D) f32
        x_nat = np.ascontiguousarray(
            xb.transpose(1, 0, 2).reshape(le, Q * D)).astype(NPBF16)
        x_t = np.ascontiguousarray(
            xb.reshape(Q, le, DC, 128).transpose(3, 2, 0, 1).reshape(128, DC * T)
        ).astype(NPBF16)

        in_maps.append({
            "x_nat": x_nat,
            "x_t": x_t,
            "uw_t": uw_t,
            "g_all": g_bf,
            "ebias": ebias.reshape(pa, 1),
        })

    nc = _get_nc(Q, le, pa)
    global LAST_NC
    LAST_NC = nc
    res = run_bass_kernel_spmd(nc, in_maps, core_ids=list(range(N_CORES)))

    out = np.zeros((B, Q, P, D), dtype=np.float32)
    for b in range(N_CORES):
        o_raw = res.results[b]["o_raw"].reshape(Q // 4, 116, D).astype(np.float64)
        sums = res.results[b]["sums8"].astype(np.float64).sum(axis=1)  # (pa,)
        Z = sums - npad_all[b] * a_pad_all[b]
        pidx = pidx_all[b]
        npi = len(pidx)
        o_q = np.empty((Q, npi, D))
        o_q[0::4] = o_raw[:, 0:npi]
        o_q[1::4] = o_raw[:, 32:32 + npi]
        o_q[2::4] = o_raw[:, 64:64 + npi]
        o_q[3::4] = o_raw[:, 96:96 + npi]
        o_n = o_q / Z[None, :npi, None]
        out[b][:, pidx, :] = o_n.astype(np.float32)
    return out


# revision 52
# speedup vs baseline: 1.0306x; 1.0306x over previous
"""Trainium2 Bass kernel for nn_AbilityGammaAttention.

Reference computation (per batch b):
    ws = s_j @ Ws_w.T + Ws_b                      # (P, A)
    uh = exp_tokens @ U_w.T                       # (Q, LE, A)
    e[q,p,t] = v . tanh(uh[q,t,:] + ws[p,:])      # (Q, P, LE)
    e masked by exp_mask (tokens), joint softmax over (Q, LE) per (b, p)
    out[q,p,:] = sum_t a[q,p,t] * exp_tokens[q,t,:], zeroed where req_mask[p]==0

Sharding: data-parallel over B across the 8 NeuronCores (batch b -> core b).

Design (v2 — separable ridge expansion instead of per-p tanh):
  The per-p ScalarE tanh over P*T*A elements (the v1 bottleneck, ~75us) is
  replaced by the separable approximation
      tanh(u + w) ~= c0(w) + cl(w)*u + sum_r cr(w)*tanh(ar*u + br)
                     + sum_j dj(w)*clamp(u, lo_j, hi_j)
  where u = uh[t,a] and w = ws[p,a].  The u-side basis is computed ONCE per
  token (R_s ScalarE tanh passes + R_v DVE clamp ops over [A, T]), and all the
  w-side structure collapses into small per-batch coefficient matrices
  G_k[a,p] = v_a * c_k(ws[p,a]) computed on the HOST (ws is host-computable
  from s_j/Ws_w).  e[:, t] then accumulates as NB matmuls [A,pa]^T @ [A,T]
  into PSUM — PE work independent of P.  The constant c0 folds into the
  per-partition exp bias.

  Other structure:
  - Host token compaction per (b,q) (as v1): unmasked tokens packed to the
    front, le = max count rounded up to 8.  Padded slots keep x=0 so they
    cannot touch the output; an additive (m-1)*1e9 rank-1 matmul masks them
    out of the softmax exactly.
  - Host req_mask compaction over p: only active p rows (padded to pa) get
    coefficients / output rows; host scatters into the zeroed full output.
  - Softmax normalization on the HOST: the device ships unnormalized
    out_raw[q,p,:] = sum_t exp(e-bound)*x and the per-chunk denominator
    accumulators (sums8); host divides.  This removes the global-Z join from
    the device pipeline, so apply/evacuation pipeline behind the main loop.
  - Everything streams in bf16 (x, xT, basis, G, a); matmuls run 1 cyc/row.
  - x is passed in BOTH layouts from the host (natural for the apply matmul,
    d-major transposed for the uh matmul) to keep PE free of transposes.
"""

import sys

if "/opt/trn_rl_repo" not in sys.path:
    sys.path.insert(0, "/opt/trn_rl_repo")

import numpy as np
import ml_dtypes

import concourse.bacc as bacc
import concourse.mybir as mybir
from concourse.masks import make_identity
from concourse.tile import TileContext

F32 = mybir.dt.float32
BF16 = mybir.dt.bfloat16
I32 = mybir.dt.int32
AF = mybir.ActivationFunctionType
ALU = mybir.AluOpType
NPBF16 = ml_dtypes.bfloat16

B, Q, LE, D, P, A = 8, 32, 128, 512, 32, 128
N_CORES = 8
DC = D // 128

# ---- ridge-basis parameters (offline fit, see session notes) -------------
# tanh(u+w) ~= c0(w) + cl(w)*u + sum_r cr(w) tanh(ALPHA_r u + BETA_r)
#            + sum_j dj(w) clamp(u, CLO_j, CHI_j)
ALPHA = [0.81614, 0.75456, 1.00156, 0.7428, 0.98255, 0.75628, 0.90041]
BETA = [-3.08039, -2.15134, -0.83113, 0.01514, 1.27361, 2.15807, 3.45762]
CLO = [-2.97443, 1.02768]
CHI = [-1.13153, 2.70644]
USE_LINEAR = True

_NG = 1201
_GRID = np.linspace(-6.5, 6.5, _NG)
_WGT = np.exp(-0.5 * _GRID**2) + 0.003


def _phi_of(grid):
    cols = [np.ones_like(grid)]
    if USE_LINEAR:
        cols.append(grid)
    for a_, b_ in zip(ALPHA, BETA):
        cols.append(np.tanh(a_ * grid + b_))
    for l_, h_ in zip(CLO, CHI):
        cols.append(np.clip(grid, l_, h_))
    return np.stack(cols, axis=0)  # (K, NG)


def _solve_matrices():
    Phi = _phi_of(_GRID)
    W = _WGT / _WGT.sum()
    Gm = (Phi * W) @ Phi.T
    Gm += 1e-9 * np.trace(Gm) / len(Gm) * np.eye(len(Gm))
    Gi = np.linalg.inv(Gm)
    M = Gi @ (Phi * W)
    phi0 = _phi_of(np.zeros(1))[:, 0]
    Kv = Gi @ phi0 / (phi0 @ Gi @ phi0)
    return M, phi0, Kv


_SOLVE_M, _PHI0, _KV = _solve_matrices()


def coeffs_for_w(w_flat):
    """c_k(w) for each w: weighted LS on the u-grid, constrained so the
    expansion is EXACT at u=0 (pads then correct on the host)."""
    Y = np.tanh(_GRID[:, None].astype(np.float32) + w_flat[None, :].astype(np.float32))
    C = _SOLVE_M.astype(np.float32) @ Y
    viol = np.tanh(w_flat.astype(np.float32)) - _PHI0.astype(np.float32) @ C
    return C + _KV.astype(np.float32)[:, None] * viol[None, :]


def build_kernel(q=Q, le=LE, pa=P):
    """Per-core kernel. q multiple of 4, le multiple of 8, pa multiple of 4."""
    T = q * le
    T2 = T // 2
    GW = 4 * le           # tokens per uh-group (4 q)
    n_t = len(ALPHA)
    n_c = len(CLO)
    NB = (1 if USE_LINEAR else 0) + n_t + n_c   # PE basis matmuls (excl mask)
    NCH = q // 4          # e-chunks (one per uh-group)
    assert le % 8 == 0 and q % 8 == 0 and pa % 4 == 0 and 4 * pa <= 128

    nc = bacc.Bacc("TRN2", target_bir_lowering=False, debug=False)

    xn_dram = nc.dram_tensor("x_nat", [le, q * D], BF16, kind="ExternalInput")
    xt_dram = nc.dram_tensor("x_t", [128, DC * T], BF16, kind="ExternalInput")
    uwt_dram = nc.dram_tensor("uw_t", [128, DC * A], BF16, kind="ExternalInput")
    g_dram = nc.dram_tensor("g_all", [A, NB * pa], BF16, kind="ExternalInput")
    eb_dram = nc.dram_tensor("ebias", [pa, 1], F32, kind="ExternalInput")
    out_dram = nc.dram_tensor("o_raw", [(q // 2) * 52, D], F32, kind="ExternalOutput")
    sums_dram = nc.dram_tensor("sums8", [pa, 2], F32, kind="ExternalOutput")

    with TileContext(nc) as tc:
        with tc.tile_pool(name="live", bufs=1) as L:
            identf = L.tile([128, 128], F32)
            identb = L.tile([128, 128], BF16)
            xn_sb = L.tile([le, q * D], BF16)
            xt_sb = L.tile([128, DC * T], BF16)
            uwt_sb = L.tile([128, DC * A], BF16)
            g_sb = L.tile([A, NB * pa], BF16)
            eb_sb = L.tile([pa, 1], F32)
            # ragged basis regions (groups per region): small leading regions
            # so the first tanh starts as early as possible
            REGS = [1, 3, 3, 1] if NCH == 8 else [1] * NCH
            RST = [sum(REGS[:i]) for i in range(len(REGS) + 1)]  # group starts
            uhq = [L.tile([A, REGS[i] * GW], BF16, name=f"uhq{i}")
                   for i in range(len(REGS))]
            e_full = L.tile([pa, T], BF16)
            aT_all = L.tile([le, q * pa], BF16)
            sums8 = L.tile([pa, 2], F32)

            # uwT first (needed by the first uh matmul)
            nc.sync.dma_start(uwt_sb[:], uwt_dram[:])

            btab = L.tile([128, n_t], F32)
            for r in range(n_t):
                nc.gpsimd.memset(btab[:, r:r + 1], float(BETA[r]))
            # 1-col warmup: hoists the ScalarE activation-table load to t~0
            wtmp = L.tile([128, 1], BF16)
            nc.scalar.activation(wtmp[:], btab[:, 0:1], AF.Tanh,
                                 bias=btab[:, 0:1], scale=1.0)
            make_identity(nc, identf)
            nc.vector.tensor_copy(identb[:], identf[:])

            with (
                tc.tile_pool(name="bas", bufs=1) as BP,
                tc.tile_pool(name="out", bufs=1) as OP,
                tc.tile_pool(name="ps", bufs=1, space="PSUM") as PS,
            ):
                # ---- load x (both layouts): all input DMAs up front -----
                # one fused multi-dim DMA per basis region (all 4 d-chunks)
                xts_v = xt_sb[:].rearrange("p (c t) -> p c t", c=DC)
                xtd_v = xt_dram.ap().rearrange("p (c t) -> p c t", c=DC)
                for ri, ng in enumerate(REGS):
                    c0, c1 = RST[ri] * GW, RST[ri + 1] * GW
                    if ri == 0:
                        for c in range(DC):
                            nc.sync.dma_start(
                                xt_sb[:, c * T + c0:c * T + c1],
                                xt_dram[:, c * T + c0:c * T + c1])
                        nc.sync.dma_start(g_sb[:], g_dram[:])
                        nc.sync.dma_start(eb_sb[:], eb_dram[:])
                    else:
                        nc.sync.dma_start(xts_v[:, :, c0:c1], xtd_v[:, :, c0:c1])
                for h in range(4):
                    nc.sync.dma_start(
                        xn_sb[:, h * (q // 4) * D:(h + 1) * (q // 4) * D],
                        xn_dram[:, h * (q // 4) * D:(h + 1) * (q // 4) * D],
                    )

                pend = []

                def flush_osb(g0, ri, osb, opss):
                    for pr in range(2):
                        if ri == len(REGS) - 1:
                            nc.scalar.activation(
                                osb[pr * 64:pr * 64 + 52, :], opss[pr][0:52, :],
                                AF.Copy, bias=0.0, scale=1.0)
                        else:
                            nc.vector.tensor_copy(
                                osb[pr * 64:pr * 64 + 52, :], opss[pr][0:52, :])
                        pi = g0 * 2 + pr
                        nc.sync.dma_start(
                            out_dram[pi * 52:(pi + 1) * 52, :],
                            osb[pr * 64:pr * 64 + 52, :])

                # region of each group, local offset within region
                reg_of = {}
                for ri, ng in enumerate(REGS):
                    for g in range(RST[ri], RST[ri + 1]):
                        reg_of[g] = (ri, (g - RST[ri]) * GW)

                # ---- uh stream: all groups before the basis/accum loop --
                for g0 in range(NCH):
                    ups = PS.tile([A, GW], F32, tag="ups", bufs=2)
                    for c in range(DC):
                        nc.tensor.matmul(
                            ups[:],
                            uwt_sb[:, c * A:(c + 1) * A],
                            xt_sb[:, c * T + g0 * GW: c * T + (g0 + 1) * GW],
                            start=(c == 0), stop=(c == DC - 1),
                        )
                    ri, lo = reg_of[g0]
                    nc.vector.tensor_copy(uhq[ri][:, lo:lo + GW], ups[:])

                # ---- u-basis per region (ScalarE tanh + DVE clamps) -----
                bts = {}
                bcs = {}
                for ri, ng in enumerate(REGS):
                    uhr = uhq[ri]
                    for r in range(n_t):
                        bt = BP.tile([A, ng * GW], BF16, tag=f"bt{ri}_{r}", bufs=1)
                        nc.scalar.activation(
                            bt[:], uhr[:], AF.Tanh,
                            bias=btab[:, r:r + 1], scale=float(ALPHA[r]),
                        )
                        bts[(ri, r)] = bt
                    for j in range(n_c):
                        bc = BP.tile([A, ng * GW], BF16, tag=f"bc{ri}_{j}", bufs=1)
                        nc.vector.tensor_scalar(
                            bc[:], uhr[:],
                            scalar1=float(CLO[j]), scalar2=float(CHI[j]),
                            op0=ALU.max, op1=ALU.min,
                        )
                        bcs[(ri, j)] = bc

                    # ---- e accum + exp + apply per 4-q chunk (1 group) --
                    for g0 in range(RST[ri], RST[ri + 1]):
                        rj, lo = reg_of[g0]
                        lsl = slice(lo, lo + GW)
                        eps = PS.tile([pa, GW], F32, tag="eps", bufs=2)
                        kb = 0
                        if USE_LINEAR:
                            nc.tensor.matmul(
                                eps[:], g_sb[:, 0:pa], uhq[rj][:, lsl],
                                start=True, stop=False,
                            )
                            kb = 1
                        for r in range(n_t):
                            nc.tensor.matmul(
                                eps[:], g_sb[:, (kb + r) * pa:(kb + r + 1) * pa],
                                bts[(rj, r)][:, lsl],
                                start=(kb == 0 and r == 0), stop=False,
                            )
                        for j in range(n_c):
                            nc.tensor.matmul(
                                eps[:],
                                g_sb[:, (kb + n_t + j) * pa:
                                     (kb + n_t + j + 1) * pa],
                                bcs[(rj, j)][:, lsl],
                                start=False, stop=(j == n_c - 1),
                            )
                        nc.scalar.activation(
                            e_full[:, g0 * GW:(g0 + 1) * GW], eps[:], AF.Exp,
                            bias=eb_sb[:, 0:1], scale=1.0,
                        )

                        # aT transposes for the 4 q of this chunk
                        atp = PS.tile([le, 4 * pa], BF16, tag="atp", bufs=1)
                        for k in range(4):
                            iq = g0 * 4 + k
                            nc.tensor.transpose(
                                atp[:, k * pa:(k + 1) * pa],
                                e_full[0:pa, iq * le:(iq + 1) * le],
                                identb[0:pa, 0:pa],
                            )
                        if ri == len(REGS) - 1:
                            nc.scalar.activation(
                                aT_all[:, g0 * 4 * pa:(g0 + 1) * 4 * pa],
                                atp[:], AF.Copy, bias=0.0, scale=1.0)
                        else:
                            nc.vector.tensor_copy(
                                aT_all[:, g0 * 4 * pa:(g0 + 1) * 4 * pa], atp[:])

                        # apply: 2 q per PSUM tile at bases {0, 32}
                        osb = OP.tile([116, D], F32, tag="osb", bufs=3)
                        opss = []
                        for pr in range(2):
                            ops = PS.tile([64, D], F32, tag="ops", bufs=3)
                            for k in range(2):
                                iq = g0 * 4 + pr * 2 + k
                                nc.tensor.matmul(
                                    ops[k * 32:k * 32 + pa, :],
                                    aT_all[:, iq * pa:(iq + 1) * pa],
                                    xn_sb[:, iq * D:(iq + 1) * D],
                                    start=True, stop=True,
                                )
                            opss.append(ops)
                        flush_osb(g0, ri, osb, opss)

                for hh in range(2):
                    nc.vector.reduce_sum(
                        sums8[:, hh:hh + 1],
                        e_full[:, hh * (T // 2):(hh + 1) * (T // 2)],
                        axis=mybir.AxisListType.X)
                nc.sync.dma_start(sums_dram[:], sums8[:])

    nc.compile()
    return nc


_NC_CACHE = {}
LAST_NC = None


def _get_nc(q=Q, le=LE, pa=P):
    key = (q, le, pa)
    if key not in _NC_CACHE:
        _NC_CACHE[key] = build_kernel(q, le, pa)
    return _NC_CACHE[key]


def _compact_tokens(exp_tokens, exp_mask, le):
    """Per-(b,q) host compaction. Returns x_c (b,q,le,D) f32 and m_c (b,q,le)."""
    b, q, full, d = exp_tokens.shape
    x_c = np.zeros((b, q, le, d), dtype=np.float32)
    m_c = np.zeros((b, q, le), dtype=np.float32)
    for bi in range(b):
        for qi in range(q):
            idx = np.flatnonzero(exp_mask[bi, qi])
            n = len(idx)
            x_c[bi, qi, :n] = exp_tokens[bi, qi, idx]
            m_c[bi, qi, :n] = 1.0
    return x_c, m_c


def kernel(exp_tokens, exp_mask, s_j, req_mask, Ws_w, Ws_b, U_w, v_w):
    """Full-input entry point: shard over B across 8 cores, gather output."""
    from concourse.bass_utils import run_bass_kernel_spmd

    exp_tokens = np.asarray(exp_tokens, dtype=np.float32)
    exp_mask = np.asarray(exp_mask, dtype=np.int32)
    s_j = np.asarray(s_j, dtype=np.float32)
    req_mask = np.asarray(req_mask, dtype=np.int32)
    Ws_w = np.asarray(Ws_w, dtype=np.float32)
    Ws_b = np.asarray(Ws_b, dtype=np.float32)
    U_w = np.asarray(U_w, dtype=np.float32)
    v_w = np.asarray(v_w, dtype=np.float32)

    counts = exp_mask.sum(axis=2)
    le = int(min(LE, max(64, -(-int(counts.max()) // 8) * 8)))
    x_c, m_c = _compact_tokens(exp_tokens, exp_mask, le)

    p_counts = req_mask.sum(axis=1)
    pa = int(min(P, max(4, -(-int(p_counts.max()) // 4) * 4)))

    bound = float(np.abs(v_w).sum()) + 1.0
    n_t, n_c = len(ALPHA), len(CLO)
    NB = (1 if USE_LINEAR else 0) + n_t + n_c

    # host-side w-branch: ws, coefficients, G matrices
    ws = (s_j.astype(np.float64) @ Ws_w.T.astype(np.float64)
          + Ws_b.astype(np.float64)).astype(np.float32)      # (B, P, A)
    vrow = v_w[0]                                            # (A,)

    T = Q * le
    # device basis values at u=0 (bf16-rounded, replicating device tiles)
    phi0_dev = np.zeros(NB, dtype=np.float32)                # excl constant
    k0 = 0
    if USE_LINEAR:
        phi0_dev[0] = 0.0
        k0 = 1
    for r in range(n_t):
        phi0_dev[k0 + r] = np.float32(np.tanh(BETA[r])).astype(NPBF16).astype(np.float32)
    for j in range(n_c):
        phi0_dev[k0 + n_t + j] = np.float32(np.clip(0.0, CLO[j], CHI[j])
                                            ).astype(NPBF16).astype(np.float32)

    uw_t = np.ascontiguousarray(
        U_w.reshape(A, DC, 128).transpose(2, 1, 0).reshape(128, DC * A)
    ).astype(NPBF16)

    in_maps = []
    pidx_all = []
    a_pad_all = []
    npad_all = []
    for b in range(N_CORES):
        pidx = np.flatnonzero(req_mask[b])
        pidx_all.append(pidx)
        ws_act = np.zeros((pa, A), dtype=np.float32)
        ws_act[:len(pidx)] = ws[b, pidx]
        C = coeffs_for_w(ws_act.reshape(-1)).reshape(-1, pa, A)  # (K, pa, A)
        # zero out padded p rows entirely
        if len(pidx) < pa:
            C[:, len(pidx):, :] = 0.0
        g_all = np.zeros((A, NB * pa), dtype=np.float32)
        for k in range(NB):
            g_all[:, k * pa:(k + 1) * pa] = (C[1 + k] * vrow[None, :]).T
        g_bf = g_all.astype(NPBF16)
        c0s = (C[0] * vrow[None, :]).sum(axis=1).astype(np.float32)  # (pa,)
        ebias = c0s - bound

        # padded slots (u = 0): their device e and exp, for host Z-correction
        gb = g_bf.astype(np.float32)
        e_pad = c0s.copy()
        for k in range(NB):
            e_pad += gb[:, k * pa:(k + 1) * pa].sum(axis=0) * phi0_dev[k]
        a_pad_all.append(np.exp(e_pad.astype(np.float64) - bound))
        npad_all.append(float(le * Q - int(m_c[b].sum())))

        xb = x_c[b]                                          # (Q, le, D) f32
        x_nat = np.ascontiguousarray(
            xb.transpose(1, 0, 2).reshape(le, Q * D)).astype(NPBF16)
        x_t = np.ascontiguousarray(
            xb.reshape(Q, le, DC, 128).transpose(3, 2, 0, 1).reshape(128, DC * T)
        ).astype(NPBF16)

        in_maps.append({
            "x_nat": x_nat,
            "x_t": x_t,
            "uw_t": uw_t,
            "g_all": g_bf,
            "ebias": ebias.reshape(pa, 1),
        })

    nc = _get_nc(Q, le, pa)
    global LAST_NC
    LAST_NC = nc
    res = run_bass_kernel_spmd(nc, in_maps, core_ids=list(range(N_CORES)))

    out = np.zeros((B, Q, P, D), dtype=np.float32)
    for b in range(N_CORES):
        o_raw = res.results[b]["o_raw"].reshape(Q // 2, 52, D).astype(np.float64)
        sums = res.results[b]["sums8"].astype(np.float64).sum(axis=1)  # (pa,)
        Z = sums - npad_all[b] * a_pad_all[b]
        pidx = pidx_all[b]
        npi = len(pidx)
        o_q = np.empty((Q, npi, D))
        o_q[0::2] = o_raw[:, 0:npi]
        o_q[1::2] = o_raw[:, 32:32 + npi]
        o_n = o_q / Z[None, :npi, None]
        out[b][:, pidx, :] = o_n.astype(np.float32)
    return out


# revision 58
# speedup vs baseline: 1.1376x; 1.1038x over previous
"""Trainium2 Bass kernel for nn_AbilityGammaAttention.

Reference computation (per batch b):
    ws = s_j @ Ws_w.T + Ws_b                      # (P, A)
    uh = exp_tokens @ U_w.T                       # (Q, LE, A)
    e[q,p,t] = v . tanh(uh[q,t,:] + ws[p,:])      # (Q, P, LE)
    e masked by exp_mask (tokens), joint softmax over (Q, LE) per (b, p)
    out[q,p,:] = sum_t a[q,p,t] * exp_tokens[q,t,:], zeroed where req_mask[p]==0

Sharding: data-parallel over B across the 8 NeuronCores (batch b -> core b).

Design (v2 — separable ridge expansion instead of per-p tanh):
  The per-p ScalarE tanh over P*T*A elements (the v1 bottleneck, ~75us) is
  replaced by the separable approximation
      tanh(u + w) ~= c0(w) + cl(w)*u + sum_r cr(w)*tanh(ar*u + br)
                     + sum_j dj(w)*clamp(u, lo_j, hi_j)
  where u = uh[t,a] and w = ws[p,a].  The u-side basis is computed ONCE per
  token (R_s ScalarE tanh passes + R_v DVE clamp ops over [A, T]), and all the
  w-side structure collapses into small per-batch coefficient matrices
  G_k[a,p] = v_a * c_k(ws[p,a]) computed on the HOST (ws is host-computable
  from s_j/Ws_w).  e[:, t] then accumulates as NB matmuls [A,pa]^T @ [A,T]
  into PSUM — PE work independent of P.  The constant c0 folds into the
  per-partition exp bias.

  Other structure:
  - Host token compaction per (b,q) (as v1): unmasked tokens packed to the
    front, le = max count rounded up to 8.  Padded slots keep x=0 so they
    cannot touch the output; an additive (m-1)*1e9 rank-1 matmul masks them
    out of the softmax exactly.
  - Host req_mask compaction over p: only active p rows (padded to pa) get
    coefficients / output rows; host scatters into the zeroed full output.
  - Softmax normalization on the HOST: the device ships unnormalized
    out_raw[q,p,:] = sum_t exp(e-bound)*x and the per-chunk denominator
    accumulators (sums8); host divides.  This removes the global-Z join from
    the device pipeline, so apply/evacuation pipeline behind the main loop.
  - Everything streams in bf16 (x, xT, basis, G, a); matmuls run 1 cyc/row.
  - x is passed in BOTH layouts from the host (natural for the apply matmul,
    d-major transposed for the uh matmul) to keep PE free of transposes.
"""

import sys

if "/opt/trn_rl_repo" not in sys.path:
    sys.path.insert(0, "/opt/trn_rl_repo")

import numpy as np
import ml_dtypes

import concourse.bacc as bacc
import concourse.mybir as mybir
from concourse.masks import make_identity
from concourse.tile import TileContext

F32 = mybir.dt.float32
BF16 = mybir.dt.bfloat16
I32 = mybir.dt.int32
AF = mybir.ActivationFunctionType
ALU = mybir.AluOpType
NPBF16 = ml_dtypes.bfloat16

B, Q, LE, D, P, A = 8, 32, 128, 512, 32, 128
N_CORES = 8
DC = D // 128

# ---- ridge-basis parameters (offline fit, see session notes) -------------
# tanh(u+w) ~= c0(w) + cl(w)*u + sum_r cr(w) tanh(ALPHA_r u + BETA_r)
#            + sum_j dj(w) clamp(u, CLO_j, CHI_j)
ALPHA = [0.79581, 0.95593, 0.62147, 0.67437, 0.93092]
BETA = [-3.04536, -2.5876, 0.06808, 1.86278, 3.57259]
CLO = [-2.22209, -1.92359, -0.50395, 0.75733]
CHI = [-0.56694, 0.10372, 1.54269, 2.25638]
USE_LINEAR = True

_NG = 1201
_GRID = np.linspace(-6.5, 6.5, _NG)
_WGT = np.exp(-0.5 * _GRID**2) + 0.003


def _phi_of(grid):
    cols = [np.ones_like(grid)]
    if USE_LINEAR:
        cols.append(grid)
    for a_, b_ in zip(ALPHA, BETA):
        cols.append(np.tanh(a_ * grid + b_))
    for l_, h_ in zip(CLO, CHI):
        cols.append(np.clip(grid, l_, h_))
    return np.stack(cols, axis=0)  # (K, NG)


def _solve_matrices():
    Phi = _phi_of(_GRID)
    W = _WGT / _WGT.sum()
    Gm = (Phi * W) @ Phi.T
    Gm += 1e-9 * np.trace(Gm) / len(Gm) * np.eye(len(Gm))
    Gi = np.linalg.inv(Gm)
    M = Gi @ (Phi * W)
    phi0 = _phi_of(np.zeros(1))[:, 0]
    Kv = Gi @ phi0 / (phi0 @ Gi @ phi0)
    return M, phi0, Kv


_SOLVE_M, _PHI0, _KV = _solve_matrices()


def coeffs_for_w(w_flat):
    """c_k(w) for each w: weighted LS on the u-grid, constrained so the
    expansion is EXACT at u=0 (pads then correct on the host)."""
    Y = np.tanh(_GRID[:, None].astype(np.float32) + w_flat[None, :].astype(np.float32))
    C = _SOLVE_M.astype(np.float32) @ Y
    viol = np.tanh(w_flat.astype(np.float32)) - _PHI0.astype(np.float32) @ C
    return C + _KV.astype(np.float32)[:, None] * viol[None, :]


def build_kernel(q=Q, le=LE, pa=P):
    """Per-core kernel. q multiple of 4, le multiple of 8, pa multiple of 4."""
    T = q * le
    T2 = T // 2
    GW = 4 * le           # tokens per uh-group (4 q)
    n_t = len(ALPHA)
    n_c = len(CLO)
    NB = (1 if USE_LINEAR else 0) + n_t + n_c   # PE basis matmuls (excl mask)
    NCH = q // 4          # e-chunks (one per uh-group)
    assert le % 8 == 0 and q % 8 == 0 and pa % 4 == 0 and 4 * pa <= 128

    nc = bacc.Bacc("TRN2", target_bir_lowering=False, debug=False)

    xn_dram = nc.dram_tensor("x_nat", [le, q * D], BF16, kind="ExternalInput")
    xt_dram = nc.dram_tensor("x_t", [128, DC * T], BF16, kind="ExternalInput")
    uwt_dram = nc.dram_tensor("uw_t", [128, DC * A], BF16, kind="ExternalInput")
    g_dram = nc.dram_tensor("g_all", [A, NB * pa], BF16, kind="ExternalInput")
    eb_dram = nc.dram_tensor("ebias", [1, pa], BF16, kind="ExternalInput")
    out_dram = nc.dram_tensor("o_raw", [(q // 2) * 52, D], F32, kind="ExternalOutput")
    sums_dram = nc.dram_tensor("sums8", [pa, 1], F32, kind="ExternalOutput")

    with TileContext(nc) as tc:
        with tc.tile_pool(name="live", bufs=1) as L:
            xn_sb = L.tile([le, q * D], BF16)
            xt_sb = L.tile([128, DC * T], BF16)
            uwt_sb = L.tile([128, DC * A], BF16)
            g_sb = L.tile([A, NB * pa], BF16)
            ebrow_sb = L.tile([1, pa], BF16)
            # ragged basis regions (groups per region): small leading regions
            # so the first tanh starts as early as possible
            REGS = [1, 3, 3, 1] if NCH == 8 else [1] * NCH
            RST = [sum(REGS[:i]) for i in range(len(REGS) + 1)]  # group starts
            uhq = [L.tile([A, REGS[i] * GW], BF16, name=f"uhq{i}")
                   for i in range(len(REGS))]
            aT_all = L.tile([le, q * pa], BF16)
            sums8 = L.tile([pa, 1], F32)

            # uwT first (needed by the first uh matmul)
            nc.sync.dma_start(uwt_sb[:], uwt_dram[:])

            btab = L.tile([128, n_t], F32)
            for r in range(n_t):
                nc.gpsimd.memset(btab[:, r:r + 1], float(BETA[r]))
            # 1-col warmup: hoists the ScalarE activation-table load to t~0
            wtmp = L.tile([128, 1], BF16)
            nc.scalar.activation(wtmp[:], btab[:, 0:1], AF.Tanh,
                                 bias=btab[:, 0:1], scale=1.0)
            onesf = L.tile([1, le], F32)
            nc.gpsimd.memset(onesf[:], 1.0)
            ones_le = L.tile([1, le], BF16)
            nc.vector.tensor_copy(ones_le[:], onesf[:])
            ocf = L.tile([le, 1], F32)
            nc.gpsimd.memset(ocf[:], 1.0)
            onecol = L.tile([le, 1], BF16)
            nc.vector.tensor_copy(onecol[:], ocf[:])
            zcol = L.tile([128, 1], F32)
            nc.gpsimd.memset(zcol[:], 0.0)

            with (
                tc.tile_pool(name="bas", bufs=1) as BP,
                tc.tile_pool(name="out", bufs=1) as OP,
                tc.tile_pool(name="ps", bufs=1, space="PSUM") as PS,
            ):
                sums_ps = PS.tile([pa, 1], F32, tag="sums", bufs=1)
                # ---- load x (both layouts): all input DMAs up front -----
                # one fused multi-dim DMA per basis region (all 4 d-chunks)
                xts_v = xt_sb[:].rearrange("p (c t) -> p c t", c=DC)
                xtd_v = xt_dram.ap().rearrange("p (c t) -> p c t", c=DC)
                for ri, ng in enumerate(REGS):
                    c0, c1 = RST[ri] * GW, RST[ri + 1] * GW
                    if ri == 0:
                        for c in range(DC):
                            nc.sync.dma_start(
                                xt_sb[:, c * T + c0:c * T + c1],
                                xt_dram[:, c * T + c0:c * T + c1])
                        nc.sync.dma_start(g_sb[:], g_dram[:])
                        nc.sync.dma_start(ebrow_sb[:], eb_dram[:])
                    else:
                        nc.sync.dma_start(xts_v[:, :, c0:c1], xtd_v[:, :, c0:c1])
                for h in range(4):
                    nc.sync.dma_start(
                        xn_sb[:, h * (q // 4) * D:(h + 1) * (q // 4) * D],
                        xn_dram[:, h * (q // 4) * D:(h + 1) * (q // 4) * D],
                    )

                pend = []

                def flush_osb(g0, ri, osb, opss):
                    for pr in range(2):
                        if ri == len(REGS) - 1:
                            nc.scalar.activation(
                                osb[pr * 64:pr * 64 + 52, :], opss[pr][0:52, :],
                                AF.Copy, bias=0.0, scale=1.0)
                        else:
                            nc.vector.tensor_copy(
                                osb[pr * 64:pr * 64 + 52, :], opss[pr][0:52, :])
                        pi = g0 * 2 + pr
                        nc.sync.dma_start(
                            out_dram[pi * 52:(pi + 1) * 52, :],
                            osb[pr * 64:pr * 64 + 52, :])

                # region of each group, local offset within region
                reg_of = {}
                for ri, ng in enumerate(REGS):
                    for g in range(RST[ri], RST[ri + 1]):
                        reg_of[g] = (ri, (g - RST[ri]) * GW)

                def emit_uh(ri):
                    for g0 in range(RST[ri], RST[ri + 1]):
                        ups = PS.tile([A, GW], F32, tag="ups", bufs=2)
                        for c in range(DC):
                            nc.tensor.matmul(
                                ups[:],
                                uwt_sb[:, c * A:(c + 1) * A],
                                xt_sb[:, c * T + g0 * GW: c * T + (g0 + 1) * GW],
                                start=(c == 0), stop=(c == DC - 1),
                            )
                        _, lo = reg_of[g0]
                        nc.vector.tensor_copy(uhq[ri][:, lo:lo + GW], ups[:])

                # ---- per region: uh (next region prefetched), basis, chunks
                emit_uh(0)
                bts = {}
                bcs = {}
                for ri, ng in enumerate(REGS):
                    if ri + 1 < len(REGS):
                        emit_uh(ri + 1)
                    uhr = uhq[ri]
                    for r in range(n_t):
                        bt = BP.tile([A, ng * GW], BF16, tag=f"bt{ri}_{r}", bufs=1)
                        nc.scalar.activation(
                            bt[:], uhr[:], AF.Tanh,
                            bias=btab[:, r:r + 1], scale=float(ALPHA[r]),
                        )
                        bts[(ri, r)] = bt
                    for j in range(n_c):
                        bc = BP.tile([A, ng * GW], BF16, tag=f"bc{ri}_{j}", bufs=1)
                        nc.vector.tensor_scalar(
                            bc[:], uhr[:],
                            scalar1=float(CLO[j]), scalar2=float(CHI[j]),
                            op0=ALU.max, op1=ALU.min,
                        )
                        bcs[(ri, j)] = bc

                    # ---- TRANSPOSED e accum + exp + apply per 4-q chunk --
                    # epsT[t, p]: basis chunks are the (128-wide) PE weights,
                    # G the 20-col moving operand -> ~4x fewer PE cycles, and
                    # exp emits aT directly (no transpose / evacuation).
                    for g0 in range(RST[ri], RST[ri + 1]):
                        rj, lo = reg_of[g0]
                        epsT = PS.tile([le, 4 * pa], F32, tag="epsT", bufs=2)
                        for k in range(4):
                            qlo = lo + k * le
                            osl = slice(k * pa, (k + 1) * pa)
                            nc.tensor.matmul(
                                epsT[:, osl], ones_le[:, 0:le],
                                ebrow_sb[:, 0:pa], start=True, stop=False,
                            )
                            kb = 0
                            if USE_LINEAR:
                                nc.tensor.matmul(
                                    epsT[:, osl], uhq[rj][:, qlo:qlo + le],
                                    g_sb[:, 0:pa], start=False, stop=False,
                                )
                                kb = 1
                            for r in range(n_t):
                                nc.tensor.matmul(
                                    epsT[:, osl],
                                    bts[(rj, r)][:, qlo:qlo + le],
                                    g_sb[:, (kb + r) * pa:(kb + r + 1) * pa],
                                    start=False, stop=False,
                                )
                            for j in range(n_c):
                                nc.tensor.matmul(
                                    epsT[:, osl],
                                    bcs[(rj, j)][:, qlo:qlo + le],
                                    g_sb[:, (kb + n_t + j) * pa:
                                         (kb + n_t + j + 1) * pa],
                                    start=False, stop=(j == n_c - 1),
                                )
                        nc.scalar.activation(
                            aT_all[:, g0 * 4 * pa:(g0 + 1) * 4 * pa], epsT[:],
                            AF.Exp, bias=zcol[0:le, 0:1], scale=1.0,
                        )

                        # apply: 2 q per PSUM tile at bases {0, 32}, plus the
                        # denominator accumulation (ones-column contraction)
                        osb = OP.tile([116, D], F32, tag="osb", bufs=3)
                        opss = []
                        for pr in range(2):
                            ops = PS.tile([64, D], F32, tag="ops", bufs=3)
                            for k in range(2):
                                iq = g0 * 4 + pr * 2 + k
                                nc.tensor.matmul(
                                    ops[k * 32:k * 32 + pa, :],
                                    aT_all[:, iq * pa:(iq + 1) * pa],
                                    xn_sb[:, iq * D:(iq + 1) * D],
                                    start=True, stop=True,
                                )
                                nc.tensor.matmul(
                                    sums_ps[:],
                                    aT_all[:, iq * pa:(iq + 1) * pa],
                                    onecol[:, 0:1],
                                    start=(iq == 0), stop=(iq == q - 1),
                                )
                            opss.append(ops)
                        flush_osb(g0, ri, osb, opss)

                nc.vector.tensor_copy(sums8[:, 0:1], sums_ps[:])
                nc.sync.dma_start(sums_dram[:], sums8[:])

    nc.compile()
    return nc


_NC_CACHE = {}
LAST_NC = None


def _get_nc(q=Q, le=LE, pa=P):
    key = (q, le, pa)
    if key not in _NC_CACHE:
        _NC_CACHE[key] = build_kernel(q, le, pa)
    return _NC_CACHE[key]


def _compact_tokens(exp_tokens, exp_mask, le):
    """Per-(b,q) host compaction. Returns x_c (b,q,le,D) f32 and m_c (b,q,le)."""
    b, q, full, d = exp_tokens.shape
    x_c = np.zeros((b, q, le, d), dtype=np.float32)
    m_c = np.zeros((b, q, le), dtype=np.float32)
    for bi in range(b):
        for qi in range(q):
            idx = np.flatnonzero(exp_mask[bi, qi])
            n = len(idx)
            x_c[bi, qi, :n] = exp_tokens[bi, qi, idx]
            m_c[bi, qi, :n] = 1.0
    return x_c, m_c


def kernel(exp_tokens, exp_mask, s_j, req_mask, Ws_w, Ws_b, U_w, v_w):
    """Full-input entry point: shard over B across 8 cores, gather output."""
    from concourse.bass_utils import run_bass_kernel_spmd

    exp_tokens = np.asarray(exp_tokens, dtype=np.float32)
    exp_mask = np.asarray(exp_mask, dtype=np.int32)
    s_j = np.asarray(s_j, dtype=np.float32)
    req_mask = np.asarray(req_mask, dtype=np.int32)
    Ws_w = np.asarray(Ws_w, dtype=np.float32)
    Ws_b = np.asarray(Ws_b, dtype=np.float32)
    U_w = np.asarray(U_w, dtype=np.float32)
    v_w = np.asarray(v_w, dtype=np.float32)

    counts = exp_mask.sum(axis=2)
    le = int(min(LE, max(64, -(-int(counts.max()) // 8) * 8)))
    x_c, m_c = _compact_tokens(exp_tokens, exp_mask, le)

    p_counts = req_mask.sum(axis=1)
    pa = int(min(P, max(4, -(-int(p_counts.max()) // 4) * 4)))

    bound = float(np.abs(v_w).sum()) + 1.0
    n_t, n_c = len(ALPHA), len(CLO)
    NB = (1 if USE_LINEAR else 0) + n_t + n_c

    # host-side w-branch: ws, coefficients, G matrices
    ws = (s_j.astype(np.float64) @ Ws_w.T.astype(np.float64)
          + Ws_b.astype(np.float64)).astype(np.float32)      # (B, P, A)
    vrow = v_w[0]                                            # (A,)

    T = Q * le
    # device basis values at u=0 (bf16-rounded, replicating device tiles)
    phi0_dev = np.zeros(NB, dtype=np.float32)                # excl constant
    k0 = 0
    if USE_LINEAR:
        phi0_dev[0] = 0.0
        k0 = 1
    for r in range(n_t):
        phi0_dev[k0 + r] = np.float32(np.tanh(BETA[r])).astype(NPBF16).astype(np.float32)
    for j in range(n_c):
        phi0_dev[k0 + n_t + j] = np.float32(np.clip(0.0, CLO[j], CHI[j])
                                            ).astype(NPBF16).astype(np.float32)

    uw_t = np.ascontiguousarray(
        U_w.reshape(A, DC, 128).transpose(2, 1, 0).reshape(128, DC * A)
    ).astype(NPBF16)

    in_maps = []
    pidx_all = []
    a_pad_all = []
    npad_all = []
    for b in range(N_CORES):
        pidx = np.flatnonzero(req_mask[b])
        pidx_all.append(pidx)
        ws_act = np.zeros((pa, A), dtype=np.float32)
        ws_act[:len(pidx)] = ws[b, pidx]
        C = coeffs_for_w(ws_act.reshape(-1)).reshape(-1, pa, A)  # (K, pa, A)
        # zero out padded p rows entirely
        if len(pidx) < pa:
            C[:, len(pidx):, :] = 0.0
        g_all = np.zeros((A, NB * pa), dtype=np.float32)
        for k in range(NB):
            g_all[:, k * pa:(k + 1) * pa] = (C[1 + k] * vrow[None, :]).T
        g_bf = g_all.astype(NPBF16)
        c0s = (C[0] * vrow[None, :]).sum(axis=1).astype(np.float32)  # (pa,)
        ebias_bf = (c0s - bound).astype(NPBF16)
        ebias_f = ebias_bf.astype(np.float32)

        # padded slots (u = 0): their device e and exp, for host Z-correction
        gb = g_bf.astype(np.float32)
        e_pad = ebias_f.copy()
        for k in range(NB):
            e_pad += gb[:, k * pa:(k + 1) * pa].sum(axis=0) * phi0_dev[k]
        a_pad_all.append(np.exp(e_pad.astype(np.float64)))
        npad_all.append(float(le * Q - int(m_c[b].sum())))

        xb = x_c[b]                                          # (Q, le, D) f32
        x_nat = np.ascontiguousarray(
            xb.transpose(1, 0, 2).reshape(le, Q * D)).astype(NPBF16)
        x_t = np.ascontiguousarray(
            xb.reshape(Q, le, DC, 128).transpose(3, 2, 0, 1).reshape(128, DC * T)
        ).astype(NPBF16)

        in_maps.append({
            "x_nat": x_nat,
            "x_t": x_t,
            "uw_t": uw_t,
            "g_all": g_bf,
            "ebias": ebias_bf.reshape(1, pa),
        })

    nc = _get_nc(Q, le, pa)
    global LAST_NC
    LAST_NC = nc
    res = run_bass_kernel_spmd(nc, in_maps, core_ids=list(range(N_CORES)))

    out = np.zeros((B, Q, P, D), dtype=np.float32)
    for b in range(N_CORES):
        o_raw = res.results[b]["o_raw"].reshape(Q // 2, 52, D).astype(np.float64)
        sums = res.results[b]["sums8"].astype(np.float64).sum(axis=1)  # (pa,)
        Z = sums - npad_all[b] * a_pad_all[b]
        pidx = pidx_all[b]
        npi = len(pidx)
        o_q = np.empty((Q, npi, D))
        o_q[0::2] = o_raw[:, 0:npi]
        o_q[1::2] = o_raw[:, 32:32 + npi]
        o_n = o_q / Z[None, :npi, None]
        out[b][:, pidx, :] = o_n.astype(np.float32)
    return out


# revision 61
# speedup vs baseline: 1.1821x; 1.0391x over previous
"""Trainium2 Bass kernel for nn_AbilityGammaAttention.

Reference computation (per batch b):
    ws = s_j @ Ws_w.T + Ws_b                      # (P, A)
    uh = exp_tokens @ U_w.T                       # (Q, LE, A)
    e[q,p,t] = v . tanh(uh[q,t,:] + ws[p,:])      # (Q, P, LE)
    e masked by exp_mask (tokens), joint softmax over (Q, LE) per (b, p)
    out[q,p,:] = sum_t a[q,p,t] * exp_tokens[q,t,:], zeroed where req_mask[p]==0

Sharding: data-parallel over B across the 8 NeuronCores (batch b -> core b).

Design (v2 — separable ridge expansion instead of per-p tanh):
  The per-p ScalarE tanh over P*T*A elements (the v1 bottleneck, ~75us) is
  replaced by the separable approximation
      tanh(u + w) ~= c0(w) + cl(w)*u + sum_r cr(w)*tanh(ar*u + br)
                     + sum_j dj(w)*clamp(u, lo_j, hi_j)
  where u = uh[t,a] and w = ws[p,a].  The u-side basis is computed ONCE per
  token (R_s ScalarE tanh passes + R_v DVE clamp ops over [A, T]), and all the
  w-side structure collapses into small per-batch coefficient matrices
  G_k[a,p] = v_a * c_k(ws[p,a]) computed on the HOST (ws is host-computable
  from s_j/Ws_w).  e[:, t] then accumulates as NB matmuls [A,pa]^T @ [A,T]
  into PSUM — PE work independent of P.  The constant c0 folds into the
  per-partition exp bias.

  Other structure:
  - Host token compaction per (b,q) (as v1): unmasked tokens packed to the
    front, le = max count rounded up to 8.  Padded slots keep x=0 so they
    cannot touch the output; an additive (m-1)*1e9 rank-1 matmul masks them
    out of the softmax exactly.
  - Host req_mask compaction over p: only active p rows (padded to pa) get
    coefficients / output rows; host scatters into the zeroed full output.
  - Softmax normalization on the HOST: the device ships unnormalized
    out_raw[q,p,:] = sum_t exp(e-bound)*x and the per-chunk denominator
    accumulators (sums8); host divides.  This removes the global-Z join from
    the device pipeline, so apply/evacuation pipeline behind the main loop.
  - Everything streams in bf16 (x, xT, basis, G, a); matmuls run 1 cyc/row.
  - x is passed in BOTH layouts from the host (natural for the apply matmul,
    d-major transposed for the uh matmul) to keep PE free of transposes.
"""

import sys

if "/opt/trn_rl_repo" not in sys.path:
    sys.path.insert(0, "/opt/trn_rl_repo")

import numpy as np
import ml_dtypes

import concourse.bacc as bacc
import concourse.mybir as mybir
from concourse.masks import make_identity
from concourse.tile import TileContext

F32 = mybir.dt.float32
BF16 = mybir.dt.bfloat16
I32 = mybir.dt.int32
AF = mybir.ActivationFunctionType
ALU = mybir.AluOpType
NPBF16 = ml_dtypes.bfloat16

B, Q, LE, D, P, A = 8, 32, 128, 512, 32, 128
N_CORES = 8
DC = D // 128

# ---- ridge-basis parameters (offline fit, see session notes) -------------
# tanh(u+w) ~= c0(w) + cl(w)*u + sum_r cr(w) tanh(ALPHA_r u + BETA_r)
#            + sum_j dj(w) clamp(u, CLO_j, CHI_j)
ALPHA = [0.79581, 0.95593, 0.62147, 0.67437, 0.93092]
BETA = [-3.04536, -2.5876, 0.06808, 1.86278, 3.57259]
CLO = [-2.22209, -1.92359, -0.50395, 0.75733]
CHI = [-0.56694, 0.10372, 1.54269, 2.25638]
USE_LINEAR = True

_NG = 1201
_GRID = np.linspace(-6.5, 6.5, _NG)
_WGT = np.exp(-0.5 * _GRID**2) + 0.003


def _phi_of(grid):
    cols = [np.ones_like(grid)]
    if USE_LINEAR:
        cols.append(grid)
    for a_, b_ in zip(ALPHA, BETA):
        cols.append(np.tanh(a_ * grid + b_))
    for l_, h_ in zip(CLO, CHI):
        cols.append(np.clip(grid, l_, h_))
    return np.stack(cols, axis=0)  # (K, NG)


def _solve_matrices():
    Phi = _phi_of(_GRID)
    W = _WGT / _WGT.sum()
    Gm = (Phi * W) @ Phi.T
    Gm += 1e-9 * np.trace(Gm) / len(Gm) * np.eye(len(Gm))
    Gi = np.linalg.inv(Gm)
    M = Gi @ (Phi * W)
    phi0 = _phi_of(np.zeros(1))[:, 0]
    Kv = Gi @ phi0 / (phi0 @ Gi @ phi0)
    return M, phi0, Kv


_SOLVE_M, _PHI0, _KV = _solve_matrices()


def coeffs_for_w(w_flat):
    """c_k(w) for each w: weighted LS on the u-grid, constrained so the
    expansion is EXACT at u=0 (pads then correct on the host)."""
    Y = np.tanh(_GRID[:, None].astype(np.float32) + w_flat[None, :].astype(np.float32))
    C = _SOLVE_M.astype(np.float32) @ Y
    viol = np.tanh(w_flat.astype(np.float32)) - _PHI0.astype(np.float32) @ C
    return C + _KV.astype(np.float32)[:, None] * viol[None, :]


def build_kernel(q=Q, le=LE, pa=P):
    """Per-core kernel. q multiple of 4, le multiple of 8, pa multiple of 4."""
    T = q * le
    T2 = T // 2
    GW = 4 * le           # tokens per uh-group (4 q)
    n_t = len(ALPHA)
    n_c = len(CLO)
    NB = (1 if USE_LINEAR else 0) + n_t + n_c   # PE basis matmuls (excl mask)
    NCH = q // 4          # e-chunks (one per uh-group)
    assert le % 8 == 0 and q % 8 == 0 and pa % 4 == 0 and 4 * pa <= 128

    nc = bacc.Bacc("TRN2", target_bir_lowering=False, debug=False)

    xn_dram = nc.dram_tensor("x_nat", [le, q * D], BF16, kind="ExternalInput")
    xt_dram = nc.dram_tensor("x_t", [128, DC * T], BF16, kind="ExternalInput")
    uwt_dram = nc.dram_tensor("uw_t", [128, DC * A], BF16, kind="ExternalInput")
    g_dram = nc.dram_tensor("g_all", [A, NB * pa], BF16, kind="ExternalInput")
    eb_dram = nc.dram_tensor("ebias", [1, pa], BF16, kind="ExternalInput")
    out_dram = nc.dram_tensor("o_raw", [(q // 2) * 52, D], F32, kind="ExternalOutput")
    sums_dram = nc.dram_tensor("sums8", [pa, 1], F32, kind="ExternalOutput")

    with TileContext(nc) as tc:
        with tc.tile_pool(name="live", bufs=1) as L:
            xn_sb = L.tile([le, q * D], BF16)
            xt_sb = L.tile([128, DC * T], BF16)
            uwt_sb = L.tile([128, DC * A], BF16)
            g_sb = L.tile([A, NB * pa], BF16)
            ebrow_sb = L.tile([1, pa], BF16)
            # ragged basis regions (groups per region): small leading regions
            # so the first tanh starts as early as possible
            REGS = [1, 3, 3, 1] if NCH == 8 else [1] * NCH
            RST = [sum(REGS[:i]) for i in range(len(REGS) + 1)]  # group starts
            uhq = [L.tile([A, REGS[i] * GW], BF16, name=f"uhq{i}")
                   for i in range(len(REGS))]
            aT_all = L.tile([le, q * pa], BF16)
            sums8 = L.tile([pa, 1], F32)

            # uwT first (needed by the first uh matmul)
            nc.sync.dma_start(uwt_sb[:], uwt_dram[:])

            zcol = L.tile([128, 1], F32)
            nc.gpsimd.memset(zcol[:], 0.0)
            btab = L.tile([128, n_t], F32)
            for r in range(n_t):
                nc.gpsimd.memset(btab[:, r:r + 1], float(BETA[r]))
            # 1-col warmup: hoists the ScalarE activation-table load to t~0
            wtmp = L.tile([128, 1], BF16)
            nc.scalar.activation(wtmp[:], btab[:, 0:1], AF.Tanh,
                                 bias=btab[:, 0:1], scale=1.0)
            onesf = L.tile([1, le], F32)
            nc.gpsimd.memset(onesf[:], 1.0)
            ones_le = L.tile([1, le], BF16)
            nc.vector.tensor_copy(ones_le[:], onesf[:])
            ocf = L.tile([le, 1], F32)
            nc.gpsimd.memset(ocf[:], 1.0)
            onecol = L.tile([le, 1], BF16)
            nc.vector.tensor_copy(onecol[:], ocf[:])

            with (
                tc.tile_pool(name="bas", bufs=1) as BP,
                tc.tile_pool(name="out", bufs=1) as OP,
                tc.tile_pool(name="ps", bufs=1, space="PSUM") as PS,
            ):
                sums_ps = PS.tile([pa, 1], F32, tag="sums", bufs=1)
                # ---- load x (both layouts): all input DMAs up front -----
                # one fused multi-dim DMA per basis region (all 4 d-chunks)
                xts_v = xt_sb[:].rearrange("p (c t) -> p c t", c=DC)
                xtd_v = xt_dram.ap().rearrange("p (c t) -> p c t", c=DC)
                for ri, ng in enumerate(REGS):
                    c0, c1 = RST[ri] * GW, RST[ri + 1] * GW
                    nc.sync.dma_start(xts_v[:, :, c0:c1], xtd_v[:, :, c0:c1])
                    if ri == 0:
                        nc.sync.dma_start(g_sb[:], g_dram[:])
                        nc.sync.dma_start(ebrow_sb[:], eb_dram[:])
                    h = ri
                    nc.sync.dma_start(
                        xn_sb[:, h * (q // 4) * D:(h + 1) * (q // 4) * D],
                        xn_dram[:, h * (q // 4) * D:(h + 1) * (q // 4) * D],
                    )

                pend = []

                def flush_osb(g0, ri, osb, opss):
                    for pr in range(2):
                        if ri >= 3:
                            nc.scalar.activation(
                                osb[pr * 64:pr * 64 + 52, :], opss[pr][0:52, :],
                                AF.Copy, bias=0.0, scale=1.0)
                        else:
                            nc.vector.tensor_copy(
                                osb[pr * 64:pr * 64 + 52, :], opss[pr][0:52, :])
                        pi = g0 * 2 + pr
                        nc.sync.dma_start(
                            out_dram[pi * 52:(pi + 1) * 52, :],
                            osb[pr * 64:pr * 64 + 52, :])

                # region of each group, local offset within region
                reg_of = {}
                for ri, ng in enumerate(REGS):
                    for g in range(RST[ri], RST[ri + 1]):
                        reg_of[g] = (ri, (g - RST[ri]) * GW)

                def emit_uh(ri):
                    for g0 in range(RST[ri], RST[ri + 1]):
                        ups = PS.tile([A, GW], F32, tag="ups", bufs=2)
                        for c in range(DC):
                            nc.tensor.matmul(
                                ups[:],
                                uwt_sb[:, c * A:(c + 1) * A],
                                xt_sb[:, c * T + g0 * GW: c * T + (g0 + 1) * GW],
                                start=(c == 0), stop=(c == DC - 1),
                            )
                        _, lo = reg_of[g0]
                        nc.vector.tensor_copy(uhq[ri][:, lo:lo + GW], ups[:])

                # ---- per region: uh (next region prefetched), basis, chunks
                emit_uh(0)
                bts = {}
                bcs = {}
                for ri, ng in enumerate(REGS):
                    if ri + 1 < len(REGS):
                        emit_uh(ri + 1)
                    uhr = uhq[ri]
                    for r in range(n_t):
                        bt = BP.tile([A, ng * GW], BF16, tag=f"bt{ri}_{r}", bufs=1)
                        nc.scalar.activation(
                            bt[:], uhr[:], AF.Tanh,
                            bias=btab[:, r:r + 1], scale=float(ALPHA[r]),
                        )
                        bts[(ri, r)] = bt
                    for j in range(n_c):
                        bc = BP.tile([A, ng * GW], BF16, tag=f"bc{ri}_{j}", bufs=1)
                        nc.vector.tensor_scalar(
                            bc[:], uhr[:],
                            scalar1=float(CLO[j]), scalar2=float(CHI[j]),
                            op0=ALU.max, op1=ALU.min,
                        )
                        bcs[(ri, j)] = bc

                    # ---- TRANSPOSED e accum + exp + apply per 4-q chunk --
                    # epsT[t, p]: basis chunks are the (128-wide) PE weights,
                    # G the 20-col moving operand -> ~4x fewer PE cycles, and
                    # exp emits aT directly (no transpose / evacuation).
                    for g0 in range(RST[ri], RST[ri + 1]):
                        rj, lo = reg_of[g0]
                        epsT = PS.tile([le, 4 * pa], F32, tag="epsT", bufs=2)
                        for k in range(4):
                            qlo = lo + k * le
                            osl = slice(k * pa, (k + 1) * pa)
                            nc.tensor.matmul(
                                epsT[:, osl], ones_le[:, 0:le],
                                ebrow_sb[:, 0:pa], start=True, stop=False,
                            )
                            kb = 0
                            if USE_LINEAR:
                                nc.tensor.matmul(
                                    epsT[:, osl], uhq[rj][:, qlo:qlo + le],
                                    g_sb[:, 0:pa], start=False, stop=False,
                                )
                                kb = 1
                            for r in range(n_t):
                                nc.tensor.matmul(
                                    epsT[:, osl],
                                    bts[(rj, r)][:, qlo:qlo + le],
                                    g_sb[:, (kb + r) * pa:(kb + r + 1) * pa],
                                    start=False, stop=False,
                                )
                            for j in range(n_c):
                                nc.tensor.matmul(
                                    epsT[:, osl],
                                    bcs[(rj, j)][:, qlo:qlo + le],
                                    g_sb[:, (kb + n_t + j) * pa:
                                         (kb + n_t + j + 1) * pa],
                                    start=False, stop=(j == n_c - 1),
                                )
                        nc.scalar.activation(
                            aT_all[:, g0 * 4 * pa:(g0 + 1) * 4 * pa], epsT[:],
                            AF.Exp, bias=zcol[0:le, 0:1], scale=1.0,
                        )

                        # apply: 2 q per PSUM tile at bases {0, 32}, plus the
                        # denominator accumulation (ones-column contraction)
                        osb = OP.tile([116, D], F32, tag="osb", bufs=3)
                        opss = []
                        for pr in range(2):
                            ops = PS.tile([64, D], F32, tag="ops", bufs=3)
                            for k in range(2):
                                iq = g0 * 4 + pr * 2 + k
                                nc.tensor.matmul(
                                    ops[k * 32:k * 32 + pa, :],
                                    aT_all[:, iq * pa:(iq + 1) * pa],
                                    xn_sb[:, iq * D:(iq + 1) * D],
                                    start=True, stop=True,
                                )
                                nc.tensor.matmul(
                                    sums_ps[:],
                                    aT_all[:, iq * pa:(iq + 1) * pa],
                                    onecol[:, 0:1],
                                    start=(iq == 0), stop=(iq == q - 1),
                                )
                            opss.append(ops)
                        flush_osb(g0, ri, osb, opss)

                nc.vector.tensor_copy(sums8[:, 0:1], sums_ps[:])
                nc.sync.dma_start(sums_dram[:], sums8[:])

    nc.compile()
    return nc


_NC_CACHE = {}
LAST_NC = None


def _get_nc(q=Q, le=LE, pa=P):
    key = (q, le, pa)
    if key not in _NC_CACHE:
        _NC_CACHE[key] = build_kernel(q, le, pa)
    return _NC_CACHE[key]


def _compact_tokens(exp_tokens, exp_mask, le):
    """Per-(b,q) host compaction. Returns x_c (b,q,le,D) f32 and m_c (b,q,le)."""
    b, q, full, d = exp_tokens.shape
    x_c = np.zeros((b, q, le, d), dtype=np.float32)
    m_c = np.zeros((b, q, le), dtype=np.float32)
    for bi in range(b):
        for qi in range(q):
            idx = np.flatnonzero(exp_mask[bi, qi])
            n = len(idx)
            x_c[bi, qi, :n] = exp_tokens[bi, qi, idx]
            m_c[bi, qi, :n] = 1.0
    return x_c, m_c


def kernel(exp_tokens, exp_mask, s_j, req_mask, Ws_w, Ws_b, U_w, v_w):
    """Full-input entry point: shard over B across 8 cores, gather output."""
    from concourse.bass_utils import run_bass_kernel_spmd

    exp_tokens = np.asarray(exp_tokens, dtype=np.float32)
    exp_mask = np.asarray(exp_mask, dtype=np.int32)
    s_j = np.asarray(s_j, dtype=np.float32)
    req_mask = np.asarray(req_mask, dtype=np.int32)
    Ws_w = np.asarray(Ws_w, dtype=np.float32)
    Ws_b = np.asarray(Ws_b, dtype=np.float32)
    U_w = np.asarray(U_w, dtype=np.float32)
    v_w = np.asarray(v_w, dtype=np.float32)

    counts = exp_mask.sum(axis=2)
    le = int(min(LE, max(64, -(-int(counts.max()) // 8) * 8)))
    x_c, m_c = _compact_tokens(exp_tokens, exp_mask, le)

    p_counts = req_mask.sum(axis=1)
    pa = int(min(P, max(4, -(-int(p_counts.max()) // 4) * 4)))

    bound = float(np.abs(v_w).sum()) + 1.0
    n_t, n_c = len(ALPHA), len(CLO)
    NB = (1 if USE_LINEAR else 0) + n_t + n_c

    # host-side w-branch: ws, coefficients, G matrices
    ws = (s_j.astype(np.float64) @ Ws_w.T.astype(np.float64)
          + Ws_b.astype(np.float64)).astype(np.float32)      # (B, P, A)
    vrow = v_w[0]                                            # (A,)

    T = Q * le
    # device basis values at u=0 (bf16-rounded, replicating device tiles)
    phi0_dev = np.zeros(NB, dtype=np.float32)                # excl constant
    k0 = 0
    if USE_LINEAR:
        phi0_dev[0] = 0.0
        k0 = 1
    for r in range(n_t):
        phi0_dev[k0 + r] = np.float32(np.tanh(BETA[r])).astype(NPBF16).astype(np.float32)
    for j in range(n_c):
        phi0_dev[k0 + n_t + j] = np.float32(np.clip(0.0, CLO[j], CHI[j])
                                            ).astype(NPBF16).astype(np.float32)

    uw_t = np.ascontiguousarray(
        U_w.reshape(A, DC, 128).transpose(2, 1, 0).reshape(128, DC * A)
    ).astype(NPBF16)

    in_maps = []
    pidx_all = []
    a_pad_all = []
    npad_all = []
    for b in range(N_CORES):
        pidx = np.flatnonzero(req_mask[b])
        pidx_all.append(pidx)
        ws_act = np.zeros((pa, A), dtype=np.float32)
        ws_act[:len(pidx)] = ws[b, pidx]
        C = coeffs_for_w(ws_act.reshape(-1)).reshape(-1, pa, A)  # (K, pa, A)
        # zero out padded p rows entirely
        if len(pidx) < pa:
            C[:, len(pidx):, :] = 0.0
        g_all = np.zeros((A, NB * pa), dtype=np.float32)
        for k in range(NB):
            g_all[:, k * pa:(k + 1) * pa] = (C[1 + k] * vrow[None, :]).T
        g_bf = g_all.astype(NPBF16)
        c0s = (C[0] * vrow[None, :]).sum(axis=1).astype(np.float32)  # (pa,)
        ebias_bf = (c0s - bound).astype(NPBF16)
        ebias_f = ebias_bf.astype(np.float32)

        # padded slots (u = 0): their device e and exp, for host Z-correction
        gb = g_bf.astype(np.float32)
        e_pad = ebias_f.copy()
        for k in range(NB):
            e_pad += gb[:, k * pa:(k + 1) * pa].sum(axis=0) * phi0_dev[k]
        a_pad_all.append(np.exp(e_pad.astype(np.float64)))
        npad_all.append(float(le * Q - int(m_c[b].sum())))

        xb = x_c[b]                                          # (Q, le, D) f32
        x_nat = np.ascontiguousarray(
            xb.transpose(1, 0, 2).reshape(le, Q * D)).astype(NPBF16)
        x_t = np.ascontiguousarray(
            xb.reshape(Q, le, DC, 128).transpose(3, 2, 0, 1).reshape(128, DC * T)
        ).astype(NPBF16)

        in_maps.append({
            "x_nat": x_nat,
            "x_t": x_t,
            "uw_t": uw_t,
            "g_all": g_bf,
            "ebias": ebias_bf.reshape(1, pa),
        })

    nc = _get_nc(Q, le, pa)
    global LAST_NC
    LAST_NC = nc
    res = run_bass_kernel_spmd(nc, in_maps, core_ids=list(range(N_CORES)))

    out = np.zeros((B, Q, P, D), dtype=np.float32)
    for b in range(N_CORES):
        o_raw = res.results[b]["o_raw"].reshape(Q // 2, 52, D).astype(np.float64)
        sums = res.results[b]["sums8"].astype(np.float64).sum(axis=1)  # (pa,)
        Z = sums - npad_all[b] * a_pad_all[b]
        pidx = pidx_all[b]
        npi = len(pidx)
        o_q = np.empty((Q, npi, D))
        o_q[0::2] = o_raw[:, 0:npi]
        o_q[1::2] = o_raw[:, 32:32 + npi]
        o_n = o_q / Z[None, :npi, None]
        out[b][:, pidx, :] = o_n.astype(np.float32)
    return out


# revision 66
# speedup vs baseline: 1.1853x; 1.0027x over previous
"""Trainium2 Bass kernel for nn_AbilityGammaAttention.

Reference computation (per batch b):
    ws = s_j @ Ws_w.T + Ws_b                      # (P, A)
    uh = exp_tokens @ U_w.T                       # (Q, LE, A)
    e[q,p,t] = v . tanh(uh[q,t,:] + ws[p,:])      # (Q, P, LE)
    e masked by exp_mask (tokens), joint softmax over (Q, LE) per (b, p)
    out[q,p,:] = sum_t a[q,p,t] * exp_tokens[q,t,:], zeroed where req_mask[p]==0

Sharding: data-parallel over B across the 8 NeuronCores (batch b -> core b).

Design (v2 — separable ridge expansion instead of per-p tanh):
  The per-p ScalarE tanh over P*T*A elements (the v1 bottleneck, ~75us) is
  replaced by the separable approximation
      tanh(u + w) ~= c0(w) + cl(w)*u + sum_r cr(w)*tanh(ar*u + br)
                     + sum_j dj(w)*clamp(u, lo_j, hi_j)
  where u = uh[t,a] and w = ws[p,a].  The u-side basis is computed ONCE per
  token (R_s ScalarE tanh passes + R_v DVE clamp ops over [A, T]), and all the
  w-side structure collapses into small per-batch coefficient matrices
  G_k[a,p] = v_a * c_k(ws[p,a]) computed on the HOST (ws is host-computable
  from s_j/Ws_w).  e[:, t] then accumulates as NB matmuls [A,pa]^T @ [A,T]
  into PSUM — PE work independent of P.  The constant c0 folds into the
  per-partition exp bias.

  Other structure:
  - Host token compaction per (b,q) (as v1): unmasked tokens packed to the
    front, le = max count rounded up to 8.  Padded slots keep x=0 so they
    cannot touch the output; an additive (m-1)*1e9 rank-1 matmul masks them
    out of the softmax exactly.
  - Host req_mask compaction over p: only active p rows (padded to pa) get
    coefficients / output rows; host scatters into the zeroed full output.
  - Softmax normalization on the HOST: the device ships unnormalized
    out_raw[q,p,:] = sum_t exp(e-bound)*x and the per-chunk denominator
    accumulators (sums8); host divides.  This removes the global-Z join from
    the device pipeline, so apply/evacuation pipeline behind the main loop.
  - Everything streams in bf16 (x, xT, basis, G, a); matmuls run 1 cyc/row.
  - x is passed in BOTH layouts from the host (natural for the apply matmul,
    d-major transposed for the uh matmul) to keep PE free of transposes.
"""

import sys

if "/opt/trn_rl_repo" not in sys.path:
    sys.path.insert(0, "/opt/trn_rl_repo")

import numpy as np
import ml_dtypes

import concourse.bacc as bacc
import concourse.mybir as mybir
from concourse.masks import make_identity
from concourse.tile import TileContext

F32 = mybir.dt.float32
BF16 = mybir.dt.bfloat16
I32 = mybir.dt.int32
AF = mybir.ActivationFunctionType
ALU = mybir.AluOpType
NPBF16 = ml_dtypes.bfloat16

B, Q, LE, D, P, A = 8, 32, 128, 512, 32, 128
N_CORES = 8
DC = D // 128

# ---- ridge-basis parameters (offline fit, see session notes) -------------
# tanh(u+w) ~= c0(w) + cl(w)*u + sum_r cr(w) tanh(ALPHA_r u + BETA_r)
#            + sum_j dj(w) clamp(u, CLO_j, CHI_j)
ALPHA = [0.79581, 0.95593, 0.62147, 0.67437, 0.93092]
BETA = [-3.04536, -2.5876, 0.06808, 1.86278, 3.57259]
CLO = [-2.22209, -1.92359, -0.50395, 0.75733]
CHI = [-0.56694, 0.10372, 1.54269, 2.25638]
USE_LINEAR = True

_NG = 1201
_GRID = np.linspace(-6.5, 6.5, _NG)
_WGT = np.exp(-0.5 * _GRID**2) + 0.003


def _phi_of(grid):
    cols = [np.ones_like(grid)]
    if USE_LINEAR:
        cols.append(grid)
    for a_, b_ in zip(ALPHA, BETA):
        cols.append(np.tanh(a_ * grid + b_))
    for l_, h_ in zip(CLO, CHI):
        cols.append(np.clip(grid, l_, h_))
    return np.stack(cols, axis=0)  # (K, NG)


def _solve_matrices():
    Phi = _phi_of(_GRID)
    W = _WGT / _WGT.sum()
    Gm = (Phi * W) @ Phi.T
    Gm += 1e-9 * np.trace(Gm) / len(Gm) * np.eye(len(Gm))
    Gi = np.linalg.inv(Gm)
    M = Gi @ (Phi * W)
    phi0 = _phi_of(np.zeros(1))[:, 0]
    Kv = Gi @ phi0 / (phi0 @ Gi @ phi0)
    return M, phi0, Kv


_SOLVE_M, _PHI0, _KV = _solve_matrices()


def coeffs_for_w(w_flat):
    """c_k(w) for each w: weighted LS on the u-grid, constrained so the
    expansion is EXACT at u=0 (pads then correct on the host)."""
    Y = np.tanh(_GRID[:, None].astype(np.float32) + w_flat[None, :].astype(np.float32))
    C = _SOLVE_M.astype(np.float32) @ Y
    viol = np.tanh(w_flat.astype(np.float32)) - _PHI0.astype(np.float32) @ C
    return C + _KV.astype(np.float32)[:, None] * viol[None, :]


def build_kernel(q=Q, le=LE, pa=P):
    """Per-core kernel. q multiple of 4, le multiple of 8, pa multiple of 4."""
    T = q * le
    T2 = T // 2
    GW = 4 * le           # tokens per uh-group (4 q)
    n_t = len(ALPHA)
    n_c = len(CLO)
    NB = (1 if USE_LINEAR else 0) + n_t + n_c   # PE basis matmuls (excl mask)
    NCH = q // 4          # e-chunks (one per uh-group)
    assert le % 8 == 0 and q % 8 == 0 and pa % 4 == 0 and 4 * pa <= 128

    nc = bacc.Bacc("TRN2", target_bir_lowering=False, debug=False)

    xn_dram = nc.dram_tensor("x_nat", [le, q * D], BF16, kind="ExternalInput")
    xt_dram = nc.dram_tensor("x_t", [128, DC * T], BF16, kind="ExternalInput")
    uwt_dram = nc.dram_tensor("uw_t", [128, DC * A], BF16, kind="ExternalInput")
    g_dram = nc.dram_tensor("g_all", [A, NB * pa], BF16, kind="ExternalInput")
    eb_dram = nc.dram_tensor("ebias", [1, pa], BF16, kind="ExternalInput")
    out_dram = nc.dram_tensor("o_raw", [(q // 2) * 52, D], F32, kind="ExternalOutput")
    sums_dram = nc.dram_tensor("sums8", [pa, 1], F32, kind="ExternalOutput")

    with TileContext(nc) as tc:
        with tc.tile_pool(name="live", bufs=1) as L:
            xn_sb = L.tile([le, q * D], BF16)
            xt_sb = L.tile([128, DC * T], BF16)
            uwt_sb = L.tile([128, DC * A], BF16)
            g_sb = L.tile([A, NB * pa], BF16)
            ebrow_sb = L.tile([1, pa], BF16)
            # ragged basis regions (groups per region): small leading regions
            # so the first tanh starts as early as possible
            REGS = [1, 3, 3, 1] if NCH == 8 else [1] * NCH
            RST = [sum(REGS[:i]) for i in range(len(REGS) + 1)]  # group starts
            uhq = [L.tile([A, REGS[i] * GW], BF16, name=f"uhq{i}")
                   for i in range(len(REGS))]
            aT_all = L.tile([le, q * pa], BF16)
            sums8 = L.tile([pa, 1], F32)

            # uwT first (needed by the first uh matmul)
            nc.sync.dma_start(uwt_sb[:], uwt_dram[:])

            zcol = L.tile([128, 1], F32)
            nc.gpsimd.memset(zcol[:], 0.0)
            btab = L.tile([128, n_t], F32)
            for r in range(n_t):
                nc.gpsimd.memset(btab[:, r:r + 1], float(BETA[r]))
            # 1-col warmup: hoists the ScalarE activation-table load to t~0
            wtmp = L.tile([128, 1], BF16)
            nc.scalar.activation(wtmp[:], btab[:, 0:1], AF.Tanh,
                                 bias=btab[:, 0:1], scale=1.0)
            onesf = L.tile([1, le], F32)
            nc.gpsimd.memset(onesf[:], 1.0)
            ones_le = L.tile([1, le], BF16)
            nc.vector.tensor_copy(ones_le[:], onesf[:])
            ocf = L.tile([le, 1], F32)
            nc.gpsimd.memset(ocf[:], 1.0)
            onecol = L.tile([le, 1], BF16)
            nc.vector.tensor_copy(onecol[:], ocf[:])

            with (
                tc.tile_pool(name="bas", bufs=1) as BP,
                tc.tile_pool(name="out", bufs=1) as OP,
                tc.tile_pool(name="ps", bufs=1, space="PSUM") as PS,
            ):
                sums_ps = PS.tile([pa, 1], F32, tag="sums", bufs=1)
                # ---- load x (both layouts): all input DMAs up front -----
                # one fused multi-dim DMA per basis region (all 4 d-chunks)
                xts_v = xt_sb[:].rearrange("p (c t) -> p c t", c=DC)
                xtd_v = xt_dram.ap().rearrange("p (c t) -> p c t", c=DC)
                for ri, ng in enumerate(REGS):
                    c0, c1 = RST[ri] * GW, RST[ri + 1] * GW
                    nc.sync.dma_start(xts_v[:, :, c0:c1], xtd_v[:, :, c0:c1])
                    if ri == 0:
                        nc.sync.dma_start(g_sb[:], g_dram[:])
                        nc.sync.dma_start(ebrow_sb[:], eb_dram[:])
                    if ri < 4:
                        h = ri
                        nc.sync.dma_start(
                            xn_sb[:, h * (q // 4) * D:(h + 1) * (q // 4) * D],
                            xn_dram[:, h * (q // 4) * D:(h + 1) * (q // 4) * D],
                        )

                pend = []

                def flush_osb(g0, ri, osb, opss):
                    for pr in range(2):
                        if ri >= 3:
                            nc.scalar.activation(
                                osb[pr * 64:pr * 64 + 52, :], opss[pr][0:52, :],
                                AF.Copy, bias=0.0, scale=1.0)
                        else:
                            nc.vector.tensor_copy(
                                osb[pr * 64:pr * 64 + 52, :], opss[pr][0:52, :])
                        pi = g0 * 2 + pr
                        nc.sync.dma_start(
                            out_dram[pi * 52:(pi + 1) * 52, :],
                            osb[pr * 64:pr * 64 + 52, :])

                # region of each group, local offset within region
                reg_of = {}
                for ri, ng in enumerate(REGS):
                    for g in range(RST[ri], RST[ri + 1]):
                        reg_of[g] = (ri, (g - RST[ri]) * GW)

                def emit_uh(ri):
                    for g0 in range(RST[ri], RST[ri + 1]):
                        ups = PS.tile([A, GW], F32, tag="ups", bufs=3)
                        for c in range(DC):
                            nc.tensor.matmul(
                                ups[:],
                                uwt_sb[:, c * A:(c + 1) * A],
                                xt_sb[:, c * T + g0 * GW: c * T + (g0 + 1) * GW],
                                start=(c == 0), stop=(c == DC - 1),
                            )
                        _, lo = reg_of[g0]
                        nc.vector.tensor_copy(uhq[ri][:, lo:lo + GW], ups[:])

                # ---- per region: uh (next region prefetched), basis, chunks
                emit_uh(0)
                bts = {}
                bcs = {}
                for ri, ng in enumerate(REGS):
                    if ri + 1 < len(REGS):
                        emit_uh(ri + 1)
                    uhr = uhq[ri]
                    for r in range(n_t):
                        bt = BP.tile([A, ng * GW], BF16, tag=f"bt{ri}_{r}", bufs=1)
                        nc.scalar.activation(
                            bt[:], uhr[:], AF.Tanh,
                            bias=btab[:, r:r + 1], scale=float(ALPHA[r]),
                        )
                        bts[(ri, r)] = bt
                    for j in range(n_c):
                        bc = BP.tile([A, ng * GW], BF16, tag=f"bc{ri}_{j}", bufs=1)
                        nc.vector.tensor_scalar(
                            bc[:], uhr[:],
                            scalar1=float(CLO[j]), scalar2=float(CHI[j]),
                            op0=ALU.max, op1=ALU.min,
                        )
                        bcs[(ri, j)] = bc

                    # ---- TRANSPOSED e accum + exp + apply per 4-q chunk --
                    # epsT[t, p]: basis chunks are the (128-wide) PE weights,
                    # G the 20-col moving operand -> ~4x fewer PE cycles, and
                    # exp emits aT directly (no transpose / evacuation).
                    for g0 in range(RST[ri], RST[ri + 1]):
                        rj, lo = reg_of[g0]
                        epsT = PS.tile([le, 4 * pa], F32, tag="epsT", bufs=2)
                        for k in range(4):
                            qlo = lo + k * le
                            osl = slice(k * pa, (k + 1) * pa)
                            nc.tensor.matmul(
                                epsT[:, osl], ones_le[:, 0:le],
                                ebrow_sb[:, 0:pa], start=True, stop=False,
                            )
                            kb = 0
                            if USE_LINEAR:
                                nc.tensor.matmul(
                                    epsT[:, osl], uhq[rj][:, qlo:qlo + le],
                                    g_sb[:, 0:pa], start=False, stop=False,
                                )
                                kb = 1
                            for r in range(n_t):
                                nc.tensor.matmul(
                                    epsT[:, osl],
                                    bts[(rj, r)][:, qlo:qlo + le],
                                    g_sb[:, (kb + r) * pa:(kb + r + 1) * pa],
                                    start=False, stop=False,
                                )
                            for j in range(n_c):
                                nc.tensor.matmul(
                                    epsT[:, osl],
                                    bcs[(rj, j)][:, qlo:qlo + le],
                                    g_sb[:, (kb + n_t + j) * pa:
                                         (kb + n_t + j + 1) * pa],
                                    start=False, stop=(j == n_c - 1),
                                )
                        nc.scalar.activation(
                            aT_all[:, g0 * 4 * pa:(g0 + 1) * 4 * pa], epsT[:],
                            AF.Exp, bias=zcol[0:le, 0:1], scale=1.0,
                        )

                        # apply: 2 q per PSUM tile at bases {0, 32}, plus the
                        # denominator accumulation (ones-column contraction)
                        osb = OP.tile([116, D], F32, tag="osb", bufs=3)
                        opss = []
                        for pr in range(2):
                            ops = PS.tile([64, D], F32, tag="ops", bufs=2)
                            for k in range(2):
                                iq = g0 * 4 + pr * 2 + k
                                nc.tensor.matmul(
                                    ops[k * 32:k * 32 + pa, :],
                                    aT_all[:, iq * pa:(iq + 1) * pa],
                                    xn_sb[:, iq * D:(iq + 1) * D],
                                    start=True, stop=True,
                                )
                                nc.tensor.matmul(
                                    sums_ps[:],
                                    aT_all[:, iq * pa:(iq + 1) * pa],
                                    onecol[:, 0:1],
                                    start=(iq == 0), stop=(iq == q - 1),
                                )
                            opss.append(ops)
                        flush_osb(g0, ri, osb, opss)

                nc.vector.tensor_copy(sums8[:, 0:1], sums_ps[:])
                nc.sync.dma_start(sums_dram[:], sums8[:])

    nc.compile()
    return nc


_NC_CACHE = {}
LAST_NC = None


def _get_nc(q=Q, le=LE, pa=P):
    key = (q, le, pa)
    if key not in _NC_CACHE:
        _NC_CACHE[key] = build_kernel(q, le, pa)
    return _NC_CACHE[key]


def _compact_tokens(exp_tokens, exp_mask, le):
    """Per-(b,q) host compaction. Returns x_c (b,q,le,D) f32 and m_c (b,q,le)."""
    b, q, full, d = exp_tokens.shape
    x_c = np.zeros((b, q, le, d), dtype=np.float32)
    m_c = np.zeros((b, q, le), dtype=np.float32)
    for bi in range(b):
        for qi in range(q):
            idx = np.flatnonzero(exp_mask[bi, qi])
            n = len(idx)
            x_c[bi, qi, :n] = exp_tokens[bi, qi, idx]
            m_c[bi, qi, :n] = 1.0
    return x_c, m_c


def kernel(exp_tokens, exp_mask, s_j, req_mask, Ws_w, Ws_b, U_w, v_w):
    """Full-input entry point: shard over B across 8 cores, gather output."""
    from concourse.bass_utils import run_bass_kernel_spmd

    exp_tokens = np.asarray(exp_tokens, dtype=np.float32)
    exp_mask = np.asarray(exp_mask, dtype=np.int32)
    s_j = np.asarray(s_j, dtype=np.float32)
    req_mask = np.asarray(req_mask, dtype=np.int32)
    Ws_w = np.asarray(Ws_w, dtype=np.float32)
    Ws_b = np.asarray(Ws_b, dtype=np.float32)
    U_w = np.asarray(U_w, dtype=np.float32)
    v_w = np.asarray(v_w, dtype=np.float32)

    counts = exp_mask.sum(axis=2)
    le = int(min(LE, max(64, -(-int(counts.max()) // 8) * 8)))
    x_c, m_c = _compact_tokens(exp_tokens, exp_mask, le)

    p_counts = req_mask.sum(axis=1)
    pa = int(min(P, max(4, -(-int(p_counts.max()) // 4) * 4)))

    bound = float(np.abs(v_w).sum()) + 1.0
    n_t, n_c = len(ALPHA), len(CLO)
    NB = (1 if USE_LINEAR else 0) + n_t + n_c

    # host-side w-branch: ws, coefficients, G matrices
    ws = (s_j.astype(np.float64) @ Ws_w.T.astype(np.float64)
          + Ws_b.astype(np.float64)).astype(np.float32)      # (B, P, A)
    vrow = v_w[0]                                            # (A,)

    T = Q * le
    # device basis values at u=0 (bf16-rounded, replicating device tiles)
    phi0_dev = np.zeros(NB, dtype=np.float32)                # excl constant
    k0 = 0
    if USE_LINEAR:
        phi0_dev[0] = 0.0
        k0 = 1
    for r in range(n_t):
        phi0_dev[k0 + r] = np.float32(np.tanh(BETA[r])).astype(NPBF16).astype(np.float32)
    for j in range(n_c):
        phi0_dev[k0 + n_t + j] = np.float32(np.clip(0.0, CLO[j], CHI[j])
                                            ).astype(NPBF16).astype(np.float32)

    uw_t = np.ascontiguousarray(
        U_w.reshape(A, DC, 128).transpose(2, 1, 0).reshape(128, DC * A)
    ).astype(NPBF16)

    in_maps = []
    pidx_all = []
    a_pad_all = []
    npad_all = []
    for b in range(N_CORES):
        pidx = np.flatnonzero(req_mask[b])
        pidx_all.append(pidx)
        ws_act = np.zeros((pa, A), dtype=np.float32)
        ws_act[:len(pidx)] = ws[b, pidx]
        C = coeffs_for_w(ws_act.reshape(-1)).reshape(-1, pa, A)  # (K, pa, A)
        # zero out padded p rows entirely
        if len(pidx) < pa:
            C[:, len(pidx):, :] = 0.0
        g_all = np.zeros((A, NB * pa), dtype=np.float32)
        for k in range(NB):
            g_all[:, k * pa:(k + 1) * pa] = (C[1 + k] * vrow[None, :]).T
        g_bf = g_all.astype(NPBF16)
        c0s = (C[0] * vrow[None, :]).sum(axis=1).astype(np.float32)  # (pa,)
        ebias_bf = (c0s - bound).astype(NPBF16)
        ebias_f = ebias_bf.astype(np.float32)

        # padded slots (u = 0): their device e and exp, for host Z-correction
        gb = g_bf.astype(np.float32)
        e_pad = ebias_f.copy()
        for k in range(NB):
            e_pad += gb[:, k * pa:(k + 1) * pa].sum(axis=0) * phi0_dev[k]
        a_pad_all.append(np.exp(e_pad.astype(np.float64)))
        npad_all.append(float(le * Q - int(m_c[b].sum())))

        xb = x_c[b]                                          # (Q, le, D) f32
        x_nat = np.ascontiguousarray(
            xb.transpose(1, 0, 2).reshape(le, Q * D)).astype(NPBF16)
        x_t = np.ascontiguousarray(
            xb.reshape(Q, le, DC, 128).transpose(3, 2, 0, 1).reshape(128, DC * T)
        ).astype(NPBF16)

        in_maps.append({
            "x_nat": x_nat,
            "x_t": x_t,
            "uw_t": uw_t,
            "g_all": g_bf,
            "ebias": ebias_bf.reshape(1, pa),
        })

    nc = _get_nc(Q, le, pa)
    global LAST_NC
    LAST_NC = nc
    res = run_bass_kernel_spmd(nc, in_maps, core_ids=list(range(N_CORES)))

    out = np.zeros((B, Q, P, D), dtype=np.float32)
    for b in range(N_CORES):
        o_raw = res.results[b]["o_raw"].reshape(Q // 2, 52, D).astype(np.float64)
        sums = res.results[b]["sums8"].astype(np.float64).sum(axis=1)  # (pa,)
        Z = sums - npad_all[b] * a_pad_all[b]
        pidx = pidx_all[b]
        npi = len(pidx)
        o_q = np.empty((Q, npi, D))
        o_q[0::2] = o_raw[:, 0:npi]
        o_q[1::2] = o_raw[:, 32:32 + npi]
        o_n = o_q / Z[None, :npi, None]
        out[b][:, pidx, :] = o_n.astype(np.float32)
    return out
